# revision 71
# baseline (speedup 1.0000x reference)
"""Distributed GQA attention (llama-style RoPE) for one TRN2 chip (8 NeuronCores).

Sharding: core c handles batch b=c//4 and sequence-quarter q=c%4 (512 q-rows).
Each core projects Q for its own rows (all 32 heads), projects K/V for its own
512 positions, AllGathers K/V within its 4-core batch group, runs attention for
its rows, and applies the output projection. Output rows are disjoint across
cores, so no all-reduce is needed; the host concatenates.

On-chip dataflow (per core):
  xT    = x.T, shipped from host pre-split into fp8 hi+lo (x16 scale)
  Q/K/V/out projections run as compensated-fp8 DoubleRow matmuls:
  3 passes (hi*hi + lo*hi + hi*lo) of half-cost fp8 matmuls == 0.75x the
  bf16 cycle count at slightly BETTER-than-bf16 accuracy.  Weight scale
  (x512) and activation scale (x16) are divided back out in the psum
  consumers (RoPE tables, v copy, the softmax ones-column, out accumulate).
  per head pair (with the previous group's out-proj interleaved):
    qT   = wq.T @ xT -> RoPE -> bf16                  [feat, sq]
    sT   = kT_h.T @ qT_h  (row-packed pairs)          [skv, sq]  psum f32
    e    = exp(sT/8) on ScalarE -> bf16
    o    = eab.T @ [v|1/16] seq-major, accumulated    [sq=4x128, 65] psum
           (col 64 collects the softmax denominator per q row; one
            start=True per psum bank, hw zero-region is bank-granular)
    o_n  = o * (16/den)  (per-partition scalar)  -> bf16 (=16*o)
    oT   = dma-xbar transpose of o_n -> [feat, sq]
  out  = oT.T @ wo (bf16), 4 head-group partials summed in f32 SBUF via
         tensor_scalar / affine_then_add with the 1/16 descale folded in.

All weights are pre-swizzled on the HOST into the exact SBUF layouts so every
DMA is a fully-linear copy.  Q/K features stay in natural interleaved order
(e0 o0 e1 o1 ...): the RoPE partner swap is an adjacent-partition
stream_shuffle on the DVE, and the cos/sin tables are shipped row-duplicated
and sign-baked as CC/SS [128, sq] with the fp8 descale folded in.
"""
import sys

sys.path.insert(0, "/opt/trn_rl_repo")

import numpy as np
import ml_dtypes
from contextlib import ExitStack

import concourse.bass as bass
import concourse.mybir as mybir
import concourse.tile as tile
from concourse import bacc
from concourse.bass_utils import run_bass_kernel_spmd
from concourse.masks import make_identity

B, S, D = 2, 2048, 2048
NQ, NKV, HD = 32, 8, 64
NCORES = 8
GPB = 4                 # cores per batch group
SQ = S // GPB           # 512 q-rows per core
P = 128
DC = D // P             # 16 contraction chunks
KF = NKV * HD           # 512 kv feature dim
KFC = KF // P           # 4 kv feature chunks
SC = S // P             # 16 skv chunks
RQ = SQ // P            # 4 q-row blocks
VW = HD + 1             # v_aug width: 64 v cols + 1 ones col (denominator)

FP = mybir.dt.float32
BF = mybir.dt.bfloat16
F8 = mybir.dt.float8e4
F8H = ml_dtypes.float8_e4m3
DR = mybir.MatmulPerfMode.DoubleRow
EXPF = mybir.ActivationFunctionType.Exp
COPYF = mybir.ActivationFunctionType.Copy
EXP_SCALE = 1.0 / 8.0   # 1/sqrt(HD)

XS = 16.0               # fp8 pre-scale on activations
WS = 512.0              # fp8 pre-scale on weights
DSC = 1.0 / (XS * WS)   # descale folded into psum consumers
ODSC = 1.0 / XS         # out-proj descale (oT carries x16; wo is bf16)


def build(solo=False):
    nc = bacc.Bacc("TRN2", target_bir_lowering=False, debug=False,
                   num_devices=1 if solo else NCORES)

    xh_e = nc.dram_tensor("xh", [P, DC, SQ], F8, kind="ExternalInput").ap()
    xl_e = nc.dram_tensor("xl", [P, DC, SQ], F8, kind="ExternalInput").ap()
    wqh_e = nc.dram_tensor("wqh", [DC, P, DC, P], F8, kind="ExternalInput").ap()
    wql_e = nc.dram_tensor("wql", [DC, P, DC, P], F8, kind="ExternalInput").ap()
    wkh_e = nc.dram_tensor("wkh", [P, DC, KF], F8, kind="ExternalInput").ap()
    wkl_e = nc.dram_tensor("wkl", [P, DC, KF], F8, kind="ExternalInput").ap()
    wvh_e = nc.dram_tensor("wvh", [P, DC, KF], F8, kind="ExternalInput").ap()
    wvl_e = nc.dram_tensor("wvl", [P, DC, KF], F8, kind="ExternalInput").ap()
    wo_e = nc.dram_tensor("wo", [RQ, RQ, P, RQ, 512], BF, kind="ExternalInput").ap()
    cc_e = nc.dram_tensor("cc", [P, SQ], FP, kind="ExternalInput").ap()
    ss_e = nc.dram_tensor("ss", [P, SQ], FP, kind="ExternalInput").ap()
    id8k_e = nc.dram_tensor("id8k", [P, P], BF, kind="ExternalInput").ap()
    out_e = nc.dram_tensor("out", [SQ, D], FP, kind="ExternalOutput").ap()

    groups = [[0, 1, 2, 3], [4, 5, 6, 7]]

    with tile.TileContext(nc) as tc, ExitStack() as ctx:
        sb = ctx.enter_context(tc.tile_pool(name="sb", bufs=1))
        rp = ctx.enter_context(tc.tile_pool(name="rp", bufs=3))
        epool = ctx.enter_context(tc.tile_pool(name="epool", bufs=6))
        npool = ctx.enter_context(tc.tile_pool(name="npool", bufs=3))
        onp = ctx.enter_context(tc.tile_pool(name="onp", bufs=3))
        opool = ctx.enter_context(tc.tile_pool(name="opool", bufs=3))
        otp = ctx.enter_context(tc.tile_pool(name="otp", bufs=2))
        early = ctx.enter_context(tc.tile_pool(name="early", bufs=1))
        wqp = ctx.enter_context(tc.tile_pool(name="wqp", bufs=3))
        dram = ctx.enter_context(tc.tile_pool(name="dram", bufs=1, space="DRAM"))
        pp = ctx.enter_context(tc.tile_pool(name="pp", bufs=2, space="PSUM"))
        psc = ctx.enter_context(tc.tile_pool(name="psc", bufs=2, space="PSUM"))
        po = ctx.enter_context(tc.tile_pool(name="po", bufs=1, space="PSUM"))

        # ---- constants ----
        cc_sb = sb.tile([P, SQ], FP)
        ss_sb = sb.tile([P, SQ], FP)
        id8k = sb.tile([P, P], BF)      # 8192 * identity (descale-matched)

        # features stay in natural interleaved order (e0 o0 e1 o1 ...): the
        # RoPE partner swap is adjacent-partition, expressible as an intra-
        # quadrant stream_shuffle; score contraction is order-invariant.
        SWAP_MASK = [i ^ 1 for i in range(32)]

        def rope_chunk(ps, dst):
            """dst = RoPE(ps)*DSC in interleaved layout; ps [128,SQ] psum."""
            t0 = rp.tile([P, SQ], FP, tag="t0")
            tsh = rp.tile([P, SQ], FP, tag="tsh")
            t1 = rp.tile([P, SQ], FP, tag="t1")
            nc.vector.tensor_mul(t0[:], ps[:], cc_sb[:])
            nc.vector.stream_shuffle(tsh[:], ps[:], SWAP_MASK)
            nc.vector.tensor_mul(t1[:], tsh[:], ss_sb[:])
            nc.vector.tensor_add(dst, t0[:], t1[:])

        qT = sb.tile([P, DC, SQ], BF)
        kag_in = dram.tile([KF, SQ], BF)
        kag_out = dram.tile([GPB * KF, SQ], BF)
        vag_in = dram.tile([SQ, KF], BF)
        vag_out = dram.tile([S, KF], BF)

        # ---- pure input loads first, split across queues: wq + x on the
        #      sync queue, the K/V/id weights on the (early-idle) scalar
        #      queue, wo on the gpsimd swdge queue later ----
        wq_tiles = {}

        def wq_load(pair):
            wh = wqp.tile([P, DC, P], F8, tag="wqh", name=f"wqh_{pair}")
            wl = wqp.tile([P, DC, P], F8, tag="wql", name=f"wql_{pair}")
            nc.sync.dma_start(wh[:], wqh_e[pair])
            nc.sync.dma_start(wl[:], wql_e[pair])
            return wh, wl

        # one queue, strict consumption order: the DMA pipe is a single
        # serialized resource in practice, so emission order = arrival order.
        # K comes FIRST so the AllGather (the longest dependency chain of
        # the attention phase) is in flight as early as possible.
        xTh = early.tile([P, DC, SQ], F8, tag="xTh", name="xTh")
        xTl = early.tile([P, DC, SQ], F8, tag="xTl", name="xTl")
        wkh = early.tile([P, DC, KF], F8, tag="wkh", name="wkh")
        wkl = early.tile([P, DC, KF], F8, tag="wkl", name="wkl")
        wvh = early.tile([P, DC, KF], F8, tag="wvh", name="wvh")
        wvl = early.tile([P, DC, KF], F8, tag="wvl", name="wvl")

        def x_load(xc):
            s4 = slice(4 * xc, 4 * (xc + 1))
            nc.sync.dma_start(xTh[:, s4, :], xh_e[:, s4, :])
            nc.sync.dma_start(xTl[:, s4, :], xl_e[:, s4, :])

        nc.sync.dma_start(wkh[:], wkh_e)
        for xc in range(4):
            s4 = slice(4 * xc, 4 * (xc + 1))
            nc.sync.dma_start(xTh[:, s4, :], xh_e[:, s4, :])
        nc.sync.dma_start(wkl[:], wkl_e)
        for xc in range(4):
            s4 = slice(4 * xc, 4 * (xc + 1))
            nc.sync.dma_start(xTl[:, s4, :], xl_e[:, s4, :])
        nc.sync.dma_start(cc_sb[:], cc_e)
        nc.sync.dma_start(ss_sb[:], ss_e)
        wq_tiles[0] = wq_load(0)
        nc.sync.dma_start(wvh[:], wvh_e)
        nc.sync.dma_start(wvl[:], wvl_e)
        wq_tiles[1] = wq_load(1)
        nc.sync.dma_start(id8k[:], id8k_e)

        def comp_passes():
            """(x, w, first) triples for the 3 compensated DR passes."""
            return ((xTh, 0, True), (xTl, 0, False), (xTh, 1, False))

        def qproj(pair):
            wh, wl = wq_tiles.pop(pair)
            qps = pp.tile([P, 512], FP, tag="pp", name="qps")
            for i in range(DC // 2):
                s2 = slice(2 * i, 2 * i + 2)
                for xs, wlo, first in comp_passes():
                    w = wl if wlo else wh
                    nc.tensor.matmul(qps[:, :SQ], lhsT=w[:, s2, :],
                                     rhs=xs[:, s2, :],
                                     start=(first and i == 0),
                                     stop=(i == DC // 2 - 1 and wlo == 1),
                                     perf_mode=DR)
            rope_chunk(qps[:, :SQ], qT[:, pair, :])

        # ---- K projection + RoPE -> AllGather (before everything else) ----
        kT_own = sb.tile([P, KFC, SQ], BF, tag="own4", name="kT_own")
        kT = early.tile([P, KFC, S], BF, tag="kT", name="kT")
        kag_v = kag_out[:].rearrange("(r c p) s -> c p r s", r=GPB, p=P)
        for fc in range(KFC):
            ps = pp.tile([P, 512], FP, tag="pp", name="kps")
            fs = slice(fc * P, (fc + 1) * P)
            # pass-major: the first pass needs only wkh + the xh chunks, so
            # the PE starts before wkl/xl even arrive
            for xs, w, first, last in ((xTh, wkh, True, False),
                                       (xTh, wkl, False, False),
                                       (xTl, wkh, False, True)):
                for i in range(DC // 2):
                    s2 = slice(2 * i, 2 * i + 2)
                    nc.tensor.matmul(ps[:, :SQ], lhsT=w[:, s2, fs],
                                     rhs=xs[:, s2, :],
                                     start=(first and i == 0),
                                     stop=(last and i == DC // 2 - 1),
                                     perf_mode=DR)
            rope_chunk(ps[:, :SQ], kT_own[:, fc, :])
            if solo:
                # per-fc gather chain: each feature chunk gathers + lands as
                # soon as its RoPE finishes, so pair 0 starts ~25us earlier
                nc.scalar.dma_start(kag_in[fc * P:(fc + 1) * P, :],
                                    kT_own[:, fc, :])
                for r in range(GPB):
                    nc.scalar.dma_start(
                        kag_out[r * KF + fc * P:r * KF + (fc + 1) * P, :],
                        kag_in[fc * P:(fc + 1) * P, :])
                nc.scalar.dma_start(
                    kT[:, fc, :].rearrange("p (r s) -> p r s", r=GPB),
                    kag_v[fc])
        if not solo:
            nc.scalar.dma_start(kag_in[:].rearrange("(c p) s -> p c s", p=P),
                                kT_own[:])
            nc.gpsimd.collective_compute(
                "AllGather", mybir.AluOpType.bypass, replica_groups=groups,
                ins=[kag_in[:]], outs=[kag_out[:]])
            for fc in range(KFC):
                nc.scalar.dma_start(
                    kT[:, fc, :].rearrange("p (r s) -> p r s", r=GPB),
                    kag_v[fc])
        qproj(0)

        # ---- V projection -> AllGather (x is the stationary side) ----
        v_own = sb.tile([P, RQ, KF], BF, tag="own4", name="v_own")
        for pc in range(RQ):
            ps = pp.tile([P, 512], FP, tag="pp", name="vps")
            pcs = slice(pc * P, (pc + 1) * P)
            for i in range(DC // 2):
                s2 = slice(2 * i, 2 * i + 2)
                for xs, wlo, first in comp_passes():
                    w = wvl if wlo else wvh
                    nc.tensor.matmul(ps[:, :KF], lhsT=xs[:, s2, pcs],
                                     rhs=w[:, s2, :],
                                     start=(first and i == 0),
                                     stop=(i == DC // 2 - 1 and wlo == 1),
                                     perf_mode=DR)
            nc.vector.tensor_scalar_mul(v_own[:, pc, :], ps[:, :KF], DSC)
            nc.sync.dma_start(vag_in[pc * P:(pc + 1) * P, :], v_own[:, pc, :])
        if solo:
            for r in range(GPB):
                nc.sync.dma_start(vag_out[r * SQ:(r + 1) * SQ, :], vag_in[:])
        else:
            nc.gpsimd.collective_compute(
                "AllGather", mybir.AluOpType.bypass, replica_groups=groups,
                ins=[vag_in[:]], outs=[vag_out[:]])

        qproj(1)

        v_aug = early.tile([P, NKV, SC, VW], BF, tag="wvh", name="v_aug")
        # only the ones-col needs the memset; the DMAs below fill cols 0:HD
        nc.gpsimd.memset(v_aug[:, :, :, HD:HD + 1], 1.0 / XS)
        for c in range(SC):
            nc.sync.dma_start(
                v_aug[:, :, c, 0:HD],
                vag_out[c * P:(c + 1) * P, :].rearrange("p (kv d) -> p kv d", d=HD))

        # ---- per-pair: Q proj + attention; prev group's out-proj interleaved ----
        oT_tiles = {}

        def wo_load(g, nf, tail=False):
            wo_nf = opool.tile([P, 4, 512], BF, tag="wo", name="wo_nf")
            # tail loads go on the swdge queue: the sync queue head-of-line
            # blocks on the last pair's transposes right then
            eng = nc.gpsimd if tail else nc.sync
            eng.dma_start(wo_nf[:], wo_e[g, nf])
            return wo_nf

        out_acc = sb.tile([P, RQ, D], FP)

        def out_proj_m(g, nf, wo_nf, m):
            """Emit one [128-row, 512-col] tile of group g's out-projection.
            Groups 0-2 accumulate (descaled by 1/16) into bf16 SBUF; group 3
            folds the running accumulator back in with a 16*I matmul, then
            the finishing descale-copy alternates ACT/DVE to halve the tail."""
            oT = oT_tiles[g]
            ms = slice(m * P, (m + 1) * P)
            ps = pp.tile([P, 512], FP, tag="pp", name="ops")
            for ch in range(4):
                nc.tensor.matmul(ps[:], lhsT=oT[:, ch, ms],
                                 rhs=wo_nf[:, ch, :],
                                 start=(ch == 0), stop=(ch == 3))
            acc = out_acc[:, m, nf * 512:(nf + 1) * 512]
            if g == 0:
                nc.vector.tensor_scalar_mul(acc, ps[:], ODSC)
            else:
                nc.vector.affine_then_add(acc, ps[:], acc, ODSC, 0.0)
            if g == 3:
                nc.gpsimd.dma_start(
                    out_e[m * P:(m + 1) * P, nf * 512:(nf + 1) * 512], acc)

        wo3_tiles = {}
        for g in range(4):                    # 4 groups of 4 pairs
            oT_tiles[g] = otp.tile([P, RQ, SQ], BF, tag="oT", name=f"oT_{g}")
            for pi in range(4):               # pairs within group
                pair = g * 4 + pi
                wo_cur = [None]
                kc = pair % 4                 # kv chunk holding both kv heads
                kva, kvb = 2 * (pair % 4), 2 * (pair % 4) + 1

                # seq-major attn.V: per head one psum bank holding 4 q-block
                # accumulators [128 q, 64 v + 1 ones]; col 64 collects the
                # softmax denominator per q row.  Exactly one start=True per
                # bank (the hw zero-region is bank-granular); every other
                # accumulator rides the same lazy zero fill.
                poA = po.tile([P, RQ, VW], FP, tag="poA", name="poA")
                poB = po.tile([P, RQ, VW], FP, tag="poB", name="poB")
                eabs = {}
                for c in range(SC + 2):
                    if c < SC:
                        # scores for both heads of the pair into one 2-bank
                        # psum tile; one exp op covers A and B
                        psAB = psc.tile([P, 1024], FP, tag="psc", name="psAB")
                        nc.tensor.matmul(psAB[:, 0:SQ],
                                         lhsT=kT[0:64, kc, c * P:(c + 1) * P],
                                         rhs=qT[0:64, pair, :],
                                         start=True, stop=True,
                                         tile_position=(0, 0))
                        nc.tensor.matmul(psAB[:, SQ:2 * SQ],
                                         lhsT=kT[64:128, kc, c * P:(c + 1) * P],
                                         rhs=qT[64:128, pair, :],
                                         start=True, stop=True,
                                         tile_position=(64, 0))
                        eab = epool.tile([P, 2, SQ], BF, tag="exp", name="eab")
                        nc.scalar.activation(eab[:], psAB[:], EXPF, scale=EXP_SCALE)
                        eabs[c] = eab
                    if c >= 2:
                        cc_ = c - 2      # attn.V lags two chunks behind exp
                        eab = eabs.pop(cc_)
                        for h, po_t, kvh in ((0, poA, kva), (1, poB, kvb)):
                            for qb in range(RQ):
                                nc.tensor.matmul(
                                    po_t[:, qb, :],
                                    lhsT=eab[:, h, qb * P:(qb + 1) * P],
                                    rhs=v_aug[:, kvh, cc_, :],
                                    start=(cc_ == 0 and qb == 0),
                                    stop=(cc_ == SC - 1),
                                    skip_group_check=True)
                    # spread next-pair q-proj and prev-group out-proj through
                    # the chunk loop so the PE never bunches them at the
                    # pair boundary (ACT rides its 1-chunk buffer)
                    if c == 1 and pair < 14:
                        wq_tiles[pair + 2] = wq_load(pair + 2)
                    if c == 3 and g >= 1:
                        wo_cur[0] = wo_load(g - 1, pi)
                    if c == 5 and pair < 14:
                        qproj(pair + 2)
                    if c == 7 and g == 3 and pi >= 2:
                        wo3_tiles[pi - 2] = wo_load(3, pi - 2)
                    if c in (9, 11, 13, 15) and g >= 1:
                        out_proj_m(g - 1, pi, wo_cur[0], (c - 9) // 2)
                # normalize (per-q denominator is a per-partition scalar;
                # the ones-col held 1/16 so rbc = 16/den and o_n = 16*o)
                # then transpose [q, (h d)] -> [(h d), q] on the DMA xbar
                o_n = onp.tile([P, RQ, 2, HD], BF, tag="on", name="o_n")
                rbcA = npool.tile([P, RQ], FP, tag="rbc", name="rbcA")
                rbcB = npool.tile([P, RQ], FP, tag="rbc", name="rbcB")
                nc.vector.reciprocal(rbcA[:], poA[:, :, HD:HD + 1])
                nc.vector.reciprocal(rbcB[:], poB[:, :, HD:HD + 1])
                for h, po_t, rbc in ((0, poA, rbcA), (1, poB, rbcB)):
                    nc.vector.tensor_mul(
                        o_n[:, :, h, :], po_t[:, :, 0:HD],
                        rbc[:].rearrange("p q -> p q ()").broadcast_to(
                            (P, RQ, HD)))
                last_pair = pair == NQ // 2 - 1
                for qb in range(RQ):
                    # scalar queue only for the last pair (ACT is done with
                    # exps there; mid-kernel it would stall exp dispatch)
                    eng = nc.scalar if last_pair and qb % 2 else nc.sync
                    eng.dma_start_transpose(
                        oT_tiles[g][:, pi, qb * P:(qb + 1) * P],
                        o_n[:, qb, :, :])

            if g == 3:
                for nf in range(4):
                    if nf + 2 < 4:
                        wo3_tiles[nf + 2] = wo_load(3, nf + 2, tail=True)
                    for m in range(RQ):
                        out_proj_m(3, nf, wo3_tiles[nf], m)

    nc.compile()
    return nc


_NC = None


def _get_nc():
    global _NC
    if _NC is None:
        _NC = build()
    return _NC


def _split8(a, scale):
    """Scaled fp8 hi/lo split: a*scale == hi + lo to ~11 mantissa bits."""
    s = (a * scale).astype(np.float32)
    hi = s.astype(F8H)
    lo = (s - hi.astype(np.float32)).astype(F8H)
    return np.ascontiguousarray(hi), np.ascontiguousarray(lo)


def _host_prep(inputs):
    """Swizzle all weights into the on-chip layouts (so device DMAs are
    linear), pre-split everything into scaled fp8 hi/lo pairs, build the
    interleaved-layout CC/SS tables, slice per-core shards.  Q/K features
    keep their natural interleaved order (e0 o0 e1 o1 ...): the RoPE partner
    swap is then an adjacent-partition stream_shuffle on the device."""
    x = np.asarray(inputs["x"], np.float32)
    cos = np.asarray(inputs["cos"], np.float32)
    sin = np.asarray(inputs["sin"], np.float32)
    wq = np.asarray(inputs["wq"], np.float32)
    wk = np.asarray(inputs["wk"], np.float32)
    wv = np.asarray(inputs["wv"], np.float32)
    wo = np.asarray(inputs["wo"], np.float32)

    # device layouts (f32, split to fp8 at the end)
    wq_dev = np.ascontiguousarray(
        wq.reshape(DC, P, DC, P).transpose(2, 1, 0, 3))
    wk_dev = np.ascontiguousarray(
        wk.reshape(DC, P, KF).transpose(1, 0, 2))
    wv_dev = np.ascontiguousarray(
        wv.reshape(DC, P, KF).transpose(1, 0, 2))
    wo_dev = np.ascontiguousarray(
        wo.reshape(RQ, RQ, P, RQ, 512).transpose(0, 3, 2, 1, 4))

    wqh, wql = _split8(wq_dev, WS)
    wkh, wkl = _split8(wk_dev, WS)
    wvh, wvl = _split8(wv_dev, WS)
    wo_bf = np.ascontiguousarray(wo_dev.astype(ml_dtypes.bfloat16))

    cosT = np.ascontiguousarray(cos.T)            # [32, S]
    sinT = np.ascontiguousarray(sin.T)
    cos2 = np.repeat(cosT, 2, axis=0)             # [64, S] rows c0 c0 c1 c1..
    sin2 = np.repeat(sinT, 2, axis=0)
    sign = np.tile(np.array([-1.0, 1.0], np.float32), 32)[:, None]
    CC = np.tile(cos2, (2, 1)) * DSC              # [128, S]; fp8 descale
    SS = np.tile(sin2 * sign, (2, 1)) * DSC

    id8k = np.ascontiguousarray(
        np.eye(P, dtype=np.float32) * XS).astype(ml_dtypes.bfloat16)

    in_maps = []
    for c in range(NCORES):
        b, q = c // GPB, c % GPB
        sl = slice(q * SQ, (q + 1) * SQ)
        x_dev = np.ascontiguousarray(
            x[b, sl, :].T.reshape(DC, P, SQ).transpose(1, 0, 2))
        xh, xl = _split8(x_dev, XS)
        in_maps.append({
            "xh": xh, "xl": xl,
            "wqh": wqh, "wql": wql, "wkh": wkh, "wkl": wkl,
            "wvh": wvh, "wvl": wvl, "wo": wo_bf,
            "cc": np.ascontiguousarray(CC[:, sl]),
            "ss": np.ascontiguousarray(SS[:, sl]),
            "id8k": id8k,
        })
    return in_maps


def kernel(**inputs):
    nc = _get_nc()
    in_maps = _host_prep(inputs)
    res = run_bass_kernel_spmd(nc, in_maps, core_ids=list(range(NCORES)))
    out = np.empty((B, S, D), np.float32)
    for c in range(NCORES):
        b, q = c // GPB, c % GPB
        out[b, q * SQ:(q + 1) * SQ, :] = res.results[c]["out"]
    return out


# revision 72
# speedup vs baseline: 1.0286x; 1.0286x over previous
"""Distributed GQA attention (llama-style RoPE) for one TRN2 chip (8 NeuronCores).

Sharding: core c handles batch b=c//4 and sequence-quarter q=c%4 (512 q-rows).
Each core projects Q for its own rows (all 32 heads), projects K/V for its own
512 positions, AllGathers K/V within its 4-core batch group, runs attention for
its rows, and applies the output projection. Output rows are disjoint across
cores, so no all-reduce is needed; the host concatenates.

On-chip dataflow (per core):
  xT    = x.T, shipped from host pre-split into fp8 hi+lo (x16 scale)
  Q/K/V/out projections run as compensated-fp8 DoubleRow matmuls:
  3 passes (hi*hi + lo*hi + hi*lo) of half-cost fp8 matmuls == 0.75x the
  bf16 cycle count at slightly BETTER-than-bf16 accuracy.  Weight scale
  (x512) and activation scale (x16) are divided back out in the psum
  consumers (RoPE tables, v copy, the softmax ones-column, out accumulate).
  per head pair (with the previous group's out-proj interleaved):
    qT   = wq.T @ xT -> RoPE -> bf16                  [feat, sq]
    sT   = kT_h.T @ qT_h  (row-packed pairs)          [skv, sq]  psum f32
    e    = exp(sT/8) on ScalarE -> bf16
    o    = eab.T @ [v|1/16] seq-major, accumulated    [sq=4x128, 65] psum
           (col 64 collects the softmax denominator per q row; one
            start=True per psum bank, hw zero-region is bank-granular)
    o_n  = o * (16/den)  (per-partition scalar)  -> bf16 (=16*o)
    oT   = dma-xbar transpose of o_n -> [feat, sq]
  out  = oT.T @ wo (bf16), 4 head-group partials summed in f32 SBUF via
         tensor_scalar / affine_then_add with the 1/16 descale folded in.

All weights are pre-swizzled on the HOST into the exact SBUF layouts so every
DMA is a fully-linear copy.  Q/K features stay in natural interleaved order
(e0 o0 e1 o1 ...): the RoPE partner swap is an adjacent-partition
stream_shuffle on the DVE, and the cos/sin tables are shipped row-duplicated
and sign-baked as CC/SS [128, sq] with the fp8 descale folded in.
"""
import sys

sys.path.insert(0, "/opt/trn_rl_repo")

import numpy as np
import ml_dtypes
from contextlib import ExitStack

import concourse.bass as bass
import concourse.mybir as mybir
import concourse.tile as tile
from concourse import bacc
from concourse.bass_utils import run_bass_kernel_spmd
from concourse.masks import make_identity

B, S, D = 2, 2048, 2048
NQ, NKV, HD = 32, 8, 64
NCORES = 8
GPB = 4                 # cores per batch group
SQ = S // GPB           # 512 q-rows per core
P = 128
DC = D // P             # 16 contraction chunks
KF = NKV * HD           # 512 kv feature dim
KFC = KF // P           # 4 kv feature chunks
SC = S // P             # 16 skv chunks
RQ = SQ // P            # 4 q-row blocks
VW = HD + 1             # v_aug width: 64 v cols + 1 ones col (denominator)

FP = mybir.dt.float32
BF = mybir.dt.bfloat16
F8 = mybir.dt.float8e4
F8H = ml_dtypes.float8_e4m3
DR = mybir.MatmulPerfMode.DoubleRow
EXPF = mybir.ActivationFunctionType.Exp
COPYF = mybir.ActivationFunctionType.Copy
EXP_SCALE = 1.0 / 8.0   # 1/sqrt(HD)

XS = 16.0               # fp8 pre-scale on activations
WS = 512.0              # fp8 pre-scale on weights
DSC = 1.0 / (XS * WS)   # descale folded into psum consumers
ODSC = 1.0 / XS         # out-proj descale (oT carries x16; wo is bf16)


def build(solo=False):
    nc = bacc.Bacc("TRN2", target_bir_lowering=False, debug=False,
                   num_devices=1 if solo else NCORES)

    xh_e = nc.dram_tensor("xh", [P, DC, SQ], F8, kind="ExternalInput").ap()
    xl_e = nc.dram_tensor("xl", [P, DC, SQ], F8, kind="ExternalInput").ap()
    wqh_e = nc.dram_tensor("wqh", [DC, P, DC, P], F8, kind="ExternalInput").ap()
    wql_e = nc.dram_tensor("wql", [DC, P, DC, P], F8, kind="ExternalInput").ap()
    wkh_e = nc.dram_tensor("wkh", [P, DC, KF], F8, kind="ExternalInput").ap()
    wkl_e = nc.dram_tensor("wkl", [P, DC, KF], F8, kind="ExternalInput").ap()
    wvh_e = nc.dram_tensor("wvh", [P, DC, KF], F8, kind="ExternalInput").ap()
    wvl_e = nc.dram_tensor("wvl", [P, DC, KF], F8, kind="ExternalInput").ap()
    wo_e = nc.dram_tensor("wo", [RQ, RQ, P, RQ, 512], BF, kind="ExternalInput").ap()
    cc_e = nc.dram_tensor("cc", [P, SQ], FP, kind="ExternalInput").ap()
    ss_e = nc.dram_tensor("ss", [P, SQ], FP, kind="ExternalInput").ap()
    id8k_e = nc.dram_tensor("id8k", [P, P], BF, kind="ExternalInput").ap()
    out_e = nc.dram_tensor("out", [SQ, D], FP, kind="ExternalOutput").ap()

    groups = [[0, 1, 2, 3], [4, 5, 6, 7]]

    with tile.TileContext(nc) as tc, ExitStack() as ctx:
        sb = ctx.enter_context(tc.tile_pool(name="sb", bufs=1))
        rp = ctx.enter_context(tc.tile_pool(name="rp", bufs=3))
        epool = ctx.enter_context(tc.tile_pool(name="epool", bufs=6))
        npool = ctx.enter_context(tc.tile_pool(name="npool", bufs=3))
        onp = ctx.enter_context(tc.tile_pool(name="onp", bufs=3))
        opool = ctx.enter_context(tc.tile_pool(name="opool", bufs=3))
        otp = ctx.enter_context(tc.tile_pool(name="otp", bufs=2))
        early = ctx.enter_context(tc.tile_pool(name="early", bufs=1))
        wqp = ctx.enter_context(tc.tile_pool(name="wqp", bufs=3))
        dram = ctx.enter_context(tc.tile_pool(name="dram", bufs=1, space="DRAM"))
        pp = ctx.enter_context(tc.tile_pool(name="pp", bufs=2, space="PSUM"))
        psc = ctx.enter_context(tc.tile_pool(name="psc", bufs=2, space="PSUM"))
        po = ctx.enter_context(tc.tile_pool(name="po", bufs=1, space="PSUM"))

        # ---- constants ----
        cc_sb = sb.tile([P, SQ], FP)
        ss_sb = sb.tile([P, SQ], FP)
        id8k = sb.tile([P, P], BF)      # 8192 * identity (descale-matched)

        # features stay in natural interleaved order (e0 o0 e1 o1 ...): the
        # RoPE partner swap is adjacent-partition, expressible as an intra-
        # quadrant stream_shuffle; score contraction is order-invariant.
        SWAP_MASK = [i ^ 1 for i in range(32)]

        def rope_chunk(ps, dst):
            """dst = RoPE(ps)*DSC in interleaved layout; ps [128,SQ] psum."""
            t0 = rp.tile([P, SQ], FP, tag="t0")
            tsh = rp.tile([P, SQ], FP, tag="tsh")
            t1 = rp.tile([P, SQ], FP, tag="t1")
            nc.vector.tensor_mul(t0[:], ps[:], cc_sb[:])
            nc.vector.stream_shuffle(tsh[:], ps[:], SWAP_MASK)
            nc.vector.tensor_mul(t1[:], tsh[:], ss_sb[:])
            nc.vector.tensor_add(dst, t0[:], t1[:])

        qT = sb.tile([P, DC, SQ], BF)
        kag_in = dram.tile([KF, SQ], BF)
        kag_out = dram.tile([GPB * KF, SQ], BF)
        vag_in = dram.tile([SQ, KF], BF)
        vag_out = dram.tile([S, KF], BF)

        # ---- pure input loads first, split across queues: wq + x on the
        #      sync queue, the K/V/id weights on the (early-idle) scalar
        #      queue, wo on the gpsimd swdge queue later ----
        wq_tiles = {}

        def wq_load(pair):
            wh = wqp.tile([P, DC, P], F8, tag="wqh", name=f"wqh_{pair}")
            wl = wqp.tile([P, DC, P], F8, tag="wql", name=f"wql_{pair}")
            nc.sync.dma_start(wh[:], wqh_e[pair])
            nc.sync.dma_start(wl[:], wql_e[pair])
            return wh, wl

        # one queue, strict consumption order: the DMA pipe is a single
        # serialized resource in practice, so emission order = arrival order.
        # K comes FIRST so the AllGather (the longest dependency chain of
        # the attention phase) is in flight as early as possible.
        xTh = early.tile([P, DC, SQ], F8, tag="xTh", name="xTh")
        xTl = early.tile([P, DC, SQ], F8, tag="xTl", name="xTl")
        wkh = early.tile([P, DC, KF], F8, tag="wkh", name="wkh")
        wkl = early.tile([P, DC, KF], F8, tag="wkl", name="wkl")
        wvh = early.tile([P, DC, KF], F8, tag="wvh", name="wvh")
        wvl = early.tile([P, DC, KF], F8, tag="wvl", name="wvl")

        def x_load(xc):
            s4 = slice(4 * xc, 4 * (xc + 1))
            nc.sync.dma_start(xTh[:, s4, :], xh_e[:, s4, :])
            nc.sync.dma_start(xTl[:, s4, :], xl_e[:, s4, :])

        nc.sync.dma_start(wkh[:], wkh_e)
        for xc in range(4):
            s4 = slice(4 * xc, 4 * (xc + 1))
            nc.sync.dma_start(xTh[:, s4, :], xh_e[:, s4, :])
        nc.sync.dma_start(wkl[:], wkl_e)
        for xc in range(4):
            s4 = slice(4 * xc, 4 * (xc + 1))
            nc.sync.dma_start(xTl[:, s4, :], xl_e[:, s4, :])
        nc.sync.dma_start(cc_sb[:], cc_e)
        nc.sync.dma_start(ss_sb[:], ss_e)
        wq_tiles[0] = wq_load(0)
        nc.sync.dma_start(wvh[:], wvh_e)
        nc.sync.dma_start(wvl[:], wvl_e)
        wq_tiles[1] = wq_load(1)
        nc.sync.dma_start(id8k[:], id8k_e)

        def comp_passes():
            """(x, w, first) triples for the 3 compensated DR passes."""
            return ((xTh, 0, True), (xTl, 0, False), (xTh, 1, False))

        def qproj(pair):
            wh, wl = wq_tiles.pop(pair)
            qps = pp.tile([P, 512], FP, tag="pp", name="qps")
            for i in range(DC // 2):
                s2 = slice(2 * i, 2 * i + 2)
                for xs, wlo, first in comp_passes():
                    w = wl if wlo else wh
                    nc.tensor.matmul(qps[:, :SQ], lhsT=w[:, s2, :],
                                     rhs=xs[:, s2, :],
                                     start=(first and i == 0),
                                     stop=(i == DC // 2 - 1 and wlo == 1),
                                     perf_mode=DR)
            rope_chunk(qps[:, :SQ], qT[:, pair, :])

        # ---- K projection + RoPE -> AllGather (before everything else) ----
        kT_own = sb.tile([P, KFC, SQ], BF, tag="own4", name="kT_own")
        for fc in range(KFC):
            ps = pp.tile([P, 512], FP, tag="pp", name="kps")
            fs = slice(fc * P, (fc + 1) * P)
            # pass-major: the first pass needs only wkh + the xh chunks, so
            # the PE starts before wkl/xl even arrive
            for xs, w, first, last in ((xTh, wkh, True, False),
                                       (xTh, wkl, False, False),
                                       (xTl, wkh, False, True)):
                for i in range(DC // 2):
                    s2 = slice(2 * i, 2 * i + 2)
                    nc.tensor.matmul(ps[:, :SQ], lhsT=w[:, s2, fs],
                                     rhs=xs[:, s2, :],
                                     start=(first and i == 0),
                                     stop=(last and i == DC // 2 - 1),
                                     perf_mode=DR)
            rope_chunk(ps[:, :SQ], kT_own[:, fc, :])
        nc.scalar.dma_start(kag_in[:].rearrange("(c p) s -> p c s", p=P),
                            kT_own[:])
        if solo:
            for r in range(GPB):
                nc.scalar.dma_start(kag_out[r * KF:(r + 1) * KF, :], kag_in[:])
        else:
            nc.gpsimd.collective_compute(
                "AllGather", mybir.AluOpType.bypass, replica_groups=groups,
                ins=[kag_in[:]], outs=[kag_out[:]])

        # ---- land gathered K (kT reuses the wk slot) while Q0 projects ----
        kT = early.tile([P, KFC, S], BF, tag="wkh", name="kT")
        kag_v = kag_out[:].rearrange("(r c p) s -> c p r s", r=GPB, p=P)
        for fc in range(KFC):
            nc.scalar.dma_start(
                kT[:, fc, :].rearrange("p (r s) -> p r s", r=GPB), kag_v[fc])
        qproj(0)

        # ---- V projection -> AllGather (x is the stationary side) ----
        v_own = sb.tile([P, RQ, KF], BF, tag="own4", name="v_own")
        for pc in range(RQ):
            ps = pp.tile([P, 512], FP, tag="pp", name="vps")
            pcs = slice(pc * P, (pc + 1) * P)
            for i in range(DC // 2):
                s2 = slice(2 * i, 2 * i + 2)
                for xs, wlo, first in comp_passes():
                    w = wvl if wlo else wvh
                    nc.tensor.matmul(ps[:, :KF], lhsT=xs[:, s2, pcs],
                                     rhs=w[:, s2, :],
                                     start=(first and i == 0),
                                     stop=(i == DC // 2 - 1 and wlo == 1),
                                     perf_mode=DR)
            nc.vector.tensor_scalar_mul(v_own[:, pc, :], ps[:, :KF], DSC)
            nc.sync.dma_start(vag_in[pc * P:(pc + 1) * P, :], v_own[:, pc, :])
        if solo:
            for r in range(GPB):
                nc.sync.dma_start(vag_out[r * SQ:(r + 1) * SQ, :], vag_in[:])
        else:
            nc.gpsimd.collective_compute(
                "AllGather", mybir.AluOpType.bypass, replica_groups=groups,
                ins=[vag_in[:]], outs=[vag_out[:]])

        qproj(1)

        v_aug = early.tile([P, NKV, SC, VW], BF, tag="wvh", name="v_aug")
        # only the ones-col needs the memset; the DMAs below fill cols 0:HD
        nc.gpsimd.memset(v_aug[:, :, :, HD:HD + 1], 1.0 / XS)
        for c in range(SC):
            nc.sync.dma_start(
                v_aug[:, :, c, 0:HD],
                vag_out[c * P:(c + 1) * P, :].rearrange("p (kv d) -> p kv d", d=HD))

        # ---- per-pair: Q proj + attention; prev group's out-proj interleaved ----
        oT_tiles = {}

        def wo_load(g, nf, tail=False):
            wo_nf = opool.tile([P, 4, 512], BF, tag="wo", name="wo_nf")
            # tail loads go on the swdge queue: the sync queue head-of-line
            # blocks on the last pair's transposes right then
            eng = nc.gpsimd if tail else nc.sync
            eng.dma_start(wo_nf[:], wo_e[g, nf])
            return wo_nf

        out_acc = sb.tile([P, RQ, D], FP)

        def out_proj_m(g, nf, wo_nf, m):
            """Emit one [128-row, 512-col] tile of group g's out-projection.
            Groups 0-2 accumulate (descaled by 1/16) into bf16 SBUF; group 3
            folds the running accumulator back in with a 16*I matmul, then
            the finishing descale-copy alternates ACT/DVE to halve the tail."""
            oT = oT_tiles[g]
            ms = slice(m * P, (m + 1) * P)
            ps = pp.tile([P, 512], FP, tag="pp", name="ops")
            for ch in range(4):
                nc.tensor.matmul(ps[:], lhsT=oT[:, ch, ms],
                                 rhs=wo_nf[:, ch, :],
                                 start=(ch == 0), stop=(ch == 3))
            acc = out_acc[:, m, nf * 512:(nf + 1) * 512]
            if g == 0:
                nc.vector.tensor_scalar_mul(acc, ps[:], ODSC)
            else:
                nc.vector.affine_then_add(acc, ps[:], acc, ODSC, 0.0)
            if g == 3:
                nc.gpsimd.dma_start(
                    out_e[m * P:(m + 1) * P, nf * 512:(nf + 1) * 512], acc)

        wo3_tiles = {}
        for g in range(4):                    # 4 groups of 4 pairs
            oT_tiles[g] = otp.tile([P, RQ, SQ], BF, tag="oT", name=f"oT_{g}")
            for pi in range(4):               # pairs within group
                pair = g * 4 + pi
                wo_cur = [None]
                kc = pair % 4                 # kv chunk holding both kv heads
                kva, kvb = 2 * (pair % 4), 2 * (pair % 4) + 1

                # seq-major attn.V: per head one psum bank holding 4 q-block
                # accumulators [128 q, 64 v + 1 ones]; col 64 collects the
                # softmax denominator per q row.  Exactly one start=True per
                # bank (the hw zero-region is bank-granular); every other
                # accumulator rides the same lazy zero fill.
                poA = po.tile([P, RQ, VW], FP, tag="poA", name="poA")
                poB = po.tile([P, RQ, VW], FP, tag="poB", name="poB")
                eabs = {}
                for c in range(SC + 2):
                    if c < SC:
                        # scores for both heads of the pair into one 2-bank
                        # psum tile; one exp op covers A and B
                        psAB = psc.tile([P, 1024], FP, tag="psc", name="psAB")
                        nc.tensor.matmul(psAB[:, 0:SQ],
                                         lhsT=kT[0:64, kc, c * P:(c + 1) * P],
                                         rhs=qT[0:64, pair, :],
                                         start=True, stop=True,
                                         tile_position=(0, 0))
                        nc.tensor.matmul(psAB[:, SQ:2 * SQ],
                                         lhsT=kT[64:128, kc, c * P:(c + 1) * P],
                                         rhs=qT[64:128, pair, :],
                                         start=True, stop=True,
                                         tile_position=(64, 0))
                        eab = epool.tile([P, 2, SQ], BF, tag="exp", name="eab")
                        nc.scalar.activation(eab[:], psAB[:], EXPF, scale=EXP_SCALE)
                        eabs[c] = eab
                    if c >= 2:
                        cc_ = c - 2      # attn.V lags two chunks behind exp
                        eab = eabs.pop(cc_)
                        for h, po_t, kvh in ((0, poA, kva), (1, poB, kvb)):
                            for qb in range(RQ):
                                nc.tensor.matmul(
                                    po_t[:, qb, :],
                                    lhsT=eab[:, h, qb * P:(qb + 1) * P],
                                    rhs=v_aug[:, kvh, cc_, :],
                                    start=(cc_ == 0 and qb == 0),
                                    stop=(cc_ == SC - 1),
                                    skip_group_check=True)
                    # spread next-pair q-proj and prev-group out-proj through
                    # the chunk loop so the PE never bunches them at the
                    # pair boundary (ACT rides its 1-chunk buffer)
                    if c == 1 and pair < 14:
                        wq_tiles[pair + 2] = wq_load(pair + 2)
                    if c == 3 and g >= 1:
                        wo_cur[0] = wo_load(g - 1, pi)
                    if c == 5 and pair < 14:
                        qproj(pair + 2)
                    if c == 7 and g == 3 and pi >= 2:
                        wo3_tiles[pi - 2] = wo_load(3, pi - 2)
                    if c in (9, 11, 13, 15) and g >= 1:
                        out_proj_m(g - 1, pi, wo_cur[0], (c - 9) // 2)
                # normalize (per-q denominator is a per-partition scalar;
                # the ones-col held 1/16 so rbc = 16/den and o_n = 16*o)
                # then transpose [q, (h d)] -> [(h d), q] on the DMA xbar
                o_n = onp.tile([P, RQ, 2, HD], BF, tag="on", name="o_n")
                rbcA = npool.tile([P, RQ], FP, tag="rbc", name="rbcA")
                rbcB = npool.tile([P, RQ], FP, tag="rbc", name="rbcB")
                nc.vector.reciprocal(rbcA[:], poA[:, :, HD:HD + 1])
                nc.vector.reciprocal(rbcB[:], poB[:, :, HD:HD + 1])
                for h, po_t, rbc in ((0, poA, rbcA), (1, poB, rbcB)):
                    nc.vector.tensor_mul(
                        o_n[:, :, h, :], po_t[:, :, 0:HD],
                        rbc[:].rearrange("p q -> p q ()").broadcast_to(
                            (P, RQ, HD)))
                last_pair = pair == NQ // 2 - 1
                for qb in range(RQ):
                    # scalar queue only for the last pair (ACT is done with
                    # exps there; mid-kernel it would stall exp dispatch)
                    eng = nc.scalar if last_pair and qb % 2 else nc.sync
                    eng.dma_start_transpose(
                        oT_tiles[g][:, pi, qb * P:(qb + 1) * P],
                        o_n[:, qb, :, :])

            if g == 3:
                for nf in range(4):
                    if nf + 2 < 4:
                        wo3_tiles[nf + 2] = wo_load(3, nf + 2, tail=True)
                    for m in range(RQ):
                        out_proj_m(3, nf, wo3_tiles[nf], m)

    nc.compile()
    return nc


_NC = None


def _get_nc():
    global _NC
    if _NC is None:
        _NC = build()
    return _NC


def _split8(a, scale):
    """Scaled fp8 hi/lo split: a*scale == hi + lo to ~11 mantissa bits."""
    s = (a * scale).astype(np.float32)
    hi = s.astype(F8H)
    lo = (s - hi.astype(np.float32)).astype(F8H)
    return np.ascontiguousarray(hi), np.ascontiguousarray(lo)


def _host_prep(inputs):
    """Swizzle all weights into the on-chip layouts (so device DMAs are
    linear), pre-split everything into scaled fp8 hi/lo pairs, build the
    interleaved-layout CC/SS tables, slice per-core shards.  Q/K features
    keep their natural interleaved order (e0 o0 e1 o1 ...): the RoPE partner
    swap is then an adjacent-partition stream_shuffle on the device."""
    x = np.asarray(inputs["x"], np.float32)
    cos = np.asarray(inputs["cos"], np.float32)
    sin = np.asarray(inputs["sin"], np.float32)
    wq = np.asarray(inputs["wq"], np.float32)
    wk = np.asarray(inputs["wk"], np.float32)
    wv = np.asarray(inputs["wv"], np.float32)
    wo = np.asarray(inputs["wo"], np.float32)

    # device layouts (f32, split to fp8 at the end)
    wq_dev = np.ascontiguousarray(
        wq.reshape(DC, P, DC, P).transpose(2, 1, 0, 3))
    wk_dev = np.ascontiguousarray(
        wk.reshape(DC, P, KF).transpose(1, 0, 2))
    wv_dev = np.ascontiguousarray(
        wv.reshape(DC, P, KF).transpose(1, 0, 2))
    wo_dev = np.ascontiguousarray(
        wo.reshape(RQ, RQ, P, RQ, 512).transpose(0, 3, 2, 1, 4))

    wqh, wql = _split8(wq_dev, WS)
    wkh, wkl = _split8(wk_dev, WS)
    wvh, wvl = _split8(wv_dev, WS)
    wo_bf = np.ascontiguousarray(wo_dev.astype(ml_dtypes.bfloat16))

    cosT = np.ascontiguousarray(cos.T)            # [32, S]
    sinT = np.ascontiguousarray(sin.T)
    cos2 = np.repeat(cosT, 2, axis=0)             # [64, S] rows c0 c0 c1 c1..
    sin2 = np.repeat(sinT, 2, axis=0)
    sign = np.tile(np.array([-1.0, 1.0], np.float32), 32)[:, None]
    CC = np.tile(cos2, (2, 1)) * DSC              # [128, S]; fp8 descale
    SS = np.tile(sin2 * sign, (2, 1)) * DSC

    id8k = np.ascontiguousarray(
        np.eye(P, dtype=np.float32) * XS).astype(ml_dtypes.bfloat16)

    in_maps = []
    for c in range(NCORES):
        b, q = c // GPB, c % GPB
        sl = slice(q * SQ, (q + 1) * SQ)
        x_dev = np.ascontiguousarray(
            x[b, sl, :].T.reshape(DC, P, SQ).transpose(1, 0, 2))
        xh, xl = _split8(x_dev, XS)
        in_maps.append({
            "xh": xh, "xl": xl,
            "wqh": wqh, "wql": wql, "wkh": wkh, "wkl": wkl,
            "wvh": wvh, "wvl": wvl, "wo": wo_bf,
            "cc": np.ascontiguousarray(CC[:, sl]),
            "ss": np.ascontiguousarray(SS[:, sl]),
            "id8k": id8k,
        })
    return in_maps


def kernel(**inputs):
    nc = _get_nc()
    in_maps = _host_prep(inputs)
    res = run_bass_kernel_spmd(nc, in_maps, core_ids=list(range(NCORES)))
    out = np.empty((B, S, D), np.float32)
    for c in range(NCORES):
        b, q = c // GPB, c % GPB
        out[b, q * SQ:(q + 1) * SQ, :] = res.results[c]["out"]
    return out


# revision 73
# speedup vs baseline: 1.0349x; 1.0061x over previous
"""Distributed GQA attention (llama-style RoPE) for one TRN2 chip (8 NeuronCores).

Sharding: core c handles batch b=c//4 and sequence-quarter q=c%4 (512 q-rows).
Each core projects Q for its own rows (all 32 heads), projects K/V for its own
512 positions, AllGathers K/V within its 4-core batch group, runs attention for
its rows, and applies the output projection. Output rows are disjoint across
cores, so no all-reduce is needed; the host concatenates.

On-chip dataflow (per core):
  xT    = x.T, shipped from host pre-split into fp8 hi+lo (x16 scale)
  Q/K/V/out projections run as compensated-fp8 DoubleRow matmuls:
  3 passes (hi*hi + lo*hi + hi*lo) of half-cost fp8 matmuls == 0.75x the
  bf16 cycle count at slightly BETTER-than-bf16 accuracy.  Weight scale
  (x512) and activation scale (x16) are divided back out in the psum
  consumers (RoPE tables, v copy, the softmax ones-column, out accumulate).
  per head pair (with the previous group's out-proj interleaved):
    qT   = wq.T @ xT -> RoPE -> bf16                  [feat, sq]
    sT   = kT_h.T @ qT_h  (row-packed pairs)          [skv, sq]  psum f32
    e    = exp(sT/8) on ScalarE -> bf16
    o    = eab.T @ [v|1/16] seq-major, accumulated    [sq=4x128, 65] psum
           (col 64 collects the softmax denominator per q row; one
            start=True per psum bank, hw zero-region is bank-granular)
    o_n  = o * (16/den)  (per-partition scalar)  -> bf16 (=16*o)
    oT   = dma-xbar transpose of o_n -> [feat, sq]
  out  = oT.T @ wo (bf16), 4 head-group partials summed in f32 SBUF via
         tensor_scalar / affine_then_add with the 1/16 descale folded in.

All weights are pre-swizzled on the HOST into the exact SBUF layouts so every
DMA is a fully-linear copy.  Q/K features stay in natural interleaved order
(e0 o0 e1 o1 ...): the RoPE partner swap is an adjacent-partition
stream_shuffle on the DVE, and the cos/sin tables are shipped row-duplicated
and sign-baked as CC/SS [128, sq] with the fp8 descale folded in.
"""
import sys

sys.path.insert(0, "/opt/trn_rl_repo")

import numpy as np
import ml_dtypes
from contextlib import ExitStack

import concourse.bass as bass
import concourse.mybir as mybir
import concourse.tile as tile
from concourse import bacc
from concourse.bass_utils import run_bass_kernel_spmd
from concourse.masks import make_identity

B, S, D = 2, 2048, 2048
NQ, NKV, HD = 32, 8, 64
NCORES = 8
GPB = 4                 # cores per batch group
SQ = S // GPB           # 512 q-rows per core
P = 128
DC = D // P             # 16 contraction chunks
KF = NKV * HD           # 512 kv feature dim
KFC = KF // P           # 4 kv feature chunks
SC = S // P             # 16 skv chunks
RQ = SQ // P            # 4 q-row blocks
VW = HD + 1             # v_aug width: 64 v cols + 1 ones col (denominator)

FP = mybir.dt.float32
BF = mybir.dt.bfloat16
F8 = mybir.dt.float8e4
F8H = ml_dtypes.float8_e4m3
DR = mybir.MatmulPerfMode.DoubleRow
EXPF = mybir.ActivationFunctionType.Exp
COPYF = mybir.ActivationFunctionType.Copy
EXP_SCALE = 1.0 / 8.0   # 1/sqrt(HD)

XS = 16.0               # fp8 pre-scale on activations
WS = 512.0              # fp8 pre-scale on weights
DSC = 1.0 / (XS * WS)   # descale folded into psum consumers
ODSC = 1.0 / XS         # out-proj descale (oT carries x16; wo is bf16)


def build(solo=False):
    nc = bacc.Bacc("TRN2", target_bir_lowering=False, debug=False,
                   num_devices=1 if solo else NCORES)

    xh_e = nc.dram_tensor("xh", [P, DC, SQ], F8, kind="ExternalInput").ap()
    xl_e = nc.dram_tensor("xl", [P, DC, SQ], F8, kind="ExternalInput").ap()
    wqh_e = nc.dram_tensor("wqh", [DC, P, DC, P], F8, kind="ExternalInput").ap()
    wql_e = nc.dram_tensor("wql", [DC, P, DC, P], F8, kind="ExternalInput").ap()
    wkh_e = nc.dram_tensor("wkh", [P, DC, KF], F8, kind="ExternalInput").ap()
    wkl_e = nc.dram_tensor("wkl", [P, DC, KF], F8, kind="ExternalInput").ap()
    wvh_e = nc.dram_tensor("wvh", [P, DC, KF], F8, kind="ExternalInput").ap()
    wvl_e = nc.dram_tensor("wvl", [P, DC, KF], F8, kind="ExternalInput").ap()
    wo_e = nc.dram_tensor("wo", [RQ, RQ, P, RQ, 512], BF, kind="ExternalInput").ap()
    cc_e = nc.dram_tensor("cc", [P, SQ], FP, kind="ExternalInput").ap()
    ss_e = nc.dram_tensor("ss", [P, SQ], FP, kind="ExternalInput").ap()
    id8k_e = nc.dram_tensor("id8k", [P, P], BF, kind="ExternalInput").ap()
    out_e = nc.dram_tensor("out", [SQ, D], FP, kind="ExternalOutput").ap()

    groups = [[0, 1, 2, 3], [4, 5, 6, 7]]

    with tile.TileContext(nc) as tc, ExitStack() as ctx:
        sb = ctx.enter_context(tc.tile_pool(name="sb", bufs=1))
        rp = ctx.enter_context(tc.tile_pool(name="rp", bufs=3))
        epool = ctx.enter_context(tc.tile_pool(name="epool", bufs=6))
        npool = ctx.enter_context(tc.tile_pool(name="npool", bufs=3))
        onp = ctx.enter_context(tc.tile_pool(name="onp", bufs=3))
        opool = ctx.enter_context(tc.tile_pool(name="opool", bufs=3))
        otp = ctx.enter_context(tc.tile_pool(name="otp", bufs=2))
        early = ctx.enter_context(tc.tile_pool(name="early", bufs=1))
        wqp = ctx.enter_context(tc.tile_pool(name="wqp", bufs=3))
        dram = ctx.enter_context(tc.tile_pool(name="dram", bufs=1, space="DRAM"))
        pp = ctx.enter_context(tc.tile_pool(name="pp", bufs=2, space="PSUM"))
        psc = ctx.enter_context(tc.tile_pool(name="psc", bufs=2, space="PSUM"))
        po = ctx.enter_context(tc.tile_pool(name="po", bufs=1, space="PSUM"))

        # ---- constants ----
        cc_sb = sb.tile([P, SQ], FP)
        ss_sb = sb.tile([P, SQ], FP)
        id8k = sb.tile([P, P], BF)      # 8192 * identity (descale-matched)

        # features stay in natural interleaved order (e0 o0 e1 o1 ...): the
        # RoPE partner swap is adjacent-partition, expressible as an intra-
        # quadrant stream_shuffle; score contraction is order-invariant.
        SWAP_MASK = [i ^ 1 for i in range(32)]

        def rope_chunk(ps, dst):
            """dst = RoPE(ps)*DSC in interleaved layout; ps [128,SQ] psum."""
            t0 = rp.tile([P, SQ], FP, tag="t0")
            tsh = rp.tile([P, SQ], FP, tag="tsh")
            t1 = rp.tile([P, SQ], FP, tag="t1")
            nc.vector.tensor_mul(t0[:], ps[:], cc_sb[:])
            nc.vector.stream_shuffle(tsh[:], ps[:], SWAP_MASK)
            nc.vector.tensor_mul(t1[:], tsh[:], ss_sb[:])
            nc.vector.tensor_add(dst, t0[:], t1[:])

        qT = sb.tile([P, DC, SQ], BF)
        kag_in = dram.tile([KF, SQ], BF)
        kag_out = dram.tile([GPB * KF, SQ], BF)
        vag_in = dram.tile([SQ, KF], BF)
        vag_out = dram.tile([S, KF], BF)

        # ---- pure input loads first, split across queues: wq + x on the
        #      sync queue, the K/V/id weights on the (early-idle) scalar
        #      queue, wo on the gpsimd swdge queue later ----
        wq_tiles = {}

        def wq_load(pair):
            wh = wqp.tile([P, DC, P], F8, tag="wqh", name=f"wqh_{pair}")
            wl = wqp.tile([P, DC, P], F8, tag="wql", name=f"wql_{pair}")
            nc.sync.dma_start(wh[:], wqh_e[pair])
            nc.sync.dma_start(wl[:], wql_e[pair])
            return wh, wl

        # one queue, strict consumption order: the DMA pipe is a single
        # serialized resource in practice, so emission order = arrival order.
        # K comes FIRST so the AllGather (the longest dependency chain of
        # the attention phase) is in flight as early as possible.
        xTh = early.tile([P, DC, SQ], F8, tag="xTh", name="xTh")
        xTl = early.tile([P, DC, SQ], F8, tag="xTl", name="xTl")
        wkh = early.tile([P, DC, KF], F8, tag="wkh", name="wkh")
        wkl = early.tile([P, DC, KF], F8, tag="wkl", name="wkl")
        wvh = early.tile([P, DC, KF], F8, tag="wvh", name="wvh")
        wvl = early.tile([P, DC, KF], F8, tag="wvl", name="wvl")

        def x_load(xc):
            s4 = slice(4 * xc, 4 * (xc + 1))
            nc.sync.dma_start(xTh[:, s4, :], xh_e[:, s4, :])
            nc.sync.dma_start(xTl[:, s4, :], xl_e[:, s4, :])

        nc.sync.dma_start(wkh[:], wkh_e)
        for xc in range(4):
            s4 = slice(4 * xc, 4 * (xc + 1))
            nc.sync.dma_start(xTh[:, s4, :], xh_e[:, s4, :])
        nc.sync.dma_start(wkl[:], wkl_e)
        for xc in range(4):
            s4 = slice(4 * xc, 4 * (xc + 1))
            nc.sync.dma_start(xTl[:, s4, :], xl_e[:, s4, :])
        nc.sync.dma_start(cc_sb[:], cc_e)
        nc.sync.dma_start(ss_sb[:], ss_e)
        wq_tiles[0] = wq_load(0)
        nc.sync.dma_start(wvh[:], wvh_e)
        nc.sync.dma_start(wvl[:], wvl_e)
        wq_tiles[1] = wq_load(1)
        nc.sync.dma_start(id8k[:], id8k_e)

        def comp_passes():
            """(x, w, first) triples for the 3 compensated DR passes."""
            return ((xTh, 0, True), (xTl, 0, False), (xTh, 1, False))

        def qproj(pair):
            wh, wl = wq_tiles.pop(pair)
            qps = pp.tile([P, 512], FP, tag="pp", name="qps")
            for i in range(DC // 2):
                s2 = slice(2 * i, 2 * i + 2)
                for xs, wlo, first in comp_passes():
                    w = wl if wlo else wh
                    nc.tensor.matmul(qps[:, :SQ], lhsT=w[:, s2, :],
                                     rhs=xs[:, s2, :],
                                     start=(first and i == 0),
                                     stop=(i == DC // 2 - 1 and wlo == 1),
                                     perf_mode=DR)
            rope_chunk(qps[:, :SQ], qT[:, pair, :])

        # ---- K projection + RoPE -> AllGather (before everything else) ----
        kT_own = sb.tile([P, KFC, SQ], BF, tag="own4", name="kT_own")
        for fc in range(KFC):
            ps = pp.tile([P, 512], FP, tag="pp", name="kps")
            fs = slice(fc * P, (fc + 1) * P)
            # pass-major: the first pass needs only wkh + the xh chunks, so
            # the PE starts before wkl/xl even arrive
            for xs, w, first, last in ((xTh, wkh, True, False),
                                       (xTh, wkl, False, False),
                                       (xTl, wkh, False, True)):
                for i in range(DC // 2):
                    s2 = slice(2 * i, 2 * i + 2)
                    nc.tensor.matmul(ps[:, :SQ], lhsT=w[:, s2, fs],
                                     rhs=xs[:, s2, :],
                                     start=(first and i == 0),
                                     stop=(last and i == DC // 2 - 1),
                                     perf_mode=DR)
            rope_chunk(ps[:, :SQ], kT_own[:, fc, :])
        nc.scalar.dma_start(kag_in[:].rearrange("(c p) s -> p c s", p=P),
                            kT_own[:])
        if solo:
            for r in range(GPB):
                nc.scalar.dma_start(kag_out[r * KF:(r + 1) * KF, :], kag_in[:])
        else:
            nc.gpsimd.collective_compute(
                "AllGather", mybir.AluOpType.bypass, replica_groups=groups,
                ins=[kag_in[:]], outs=[kag_out[:]])

        # ---- land gathered K (kT reuses the wk slot) while Q0 projects ----
        kT = early.tile([P, KFC, S], BF, tag="wkh", name="kT")
        kag_v = kag_out[:].rearrange("(r c p) s -> c p r s", r=GPB, p=P)
        for fc in range(KFC):
            nc.scalar.dma_start(
                kT[:, fc, :].rearrange("p (r s) -> p r s", r=GPB), kag_v[fc])
        qproj(0)

        # ---- V projection -> AllGather (x is the stationary side) ----
        v_own = sb.tile([P, RQ, KF], BF, tag="own4", name="v_own")
        for pc in range(RQ):
            ps = pp.tile([P, 512], FP, tag="pp", name="vps")
            pcs = slice(pc * P, (pc + 1) * P)
            for i in range(DC // 2):
                s2 = slice(2 * i, 2 * i + 2)
                for xs, wlo, first in comp_passes():
                    w = wvl if wlo else wvh
                    nc.tensor.matmul(ps[:, :KF], lhsT=xs[:, s2, pcs],
                                     rhs=w[:, s2, :],
                                     start=(first and i == 0),
                                     stop=(i == DC // 2 - 1 and wlo == 1),
                                     perf_mode=DR)
            nc.vector.tensor_scalar_mul(v_own[:, pc, :], ps[:, :KF], DSC)
            nc.sync.dma_start(vag_in[pc * P:(pc + 1) * P, :], v_own[:, pc, :])
        if solo:
            for r in range(GPB):
                nc.sync.dma_start(vag_out[r * SQ:(r + 1) * SQ, :], vag_in[:])
        else:
            nc.gpsimd.collective_compute(
                "AllGather", mybir.AluOpType.bypass, replica_groups=groups,
                ins=[vag_in[:]], outs=[vag_out[:]])

        qproj(1)

        v_aug = early.tile([P, NKV, SC, VW], BF, tag="wvh", name="v_aug")
        # only the ones-col needs the memset; the DMAs below fill cols 0:HD
        nc.gpsimd.memset(v_aug[:, :, :, HD:HD + 1], 1.0 / XS)
        for c in range(SC):
            nc.sync.dma_start(
                v_aug[:, :, c, 0:HD],
                vag_out[c * P:(c + 1) * P, :].rearrange("p (kv d) -> p kv d", d=HD))

        # ---- per-pair: Q proj + attention; prev group's out-proj interleaved ----
        oT_tiles = {}

        def wo_load(g, nf, tail=False):
            wo_nf = opool.tile([P, 4, 512], BF, tag="wo", name="wo_nf")
            # tail loads go on the swdge queue: the sync queue head-of-line
            # blocks on the last pair's transposes right then
            eng = nc.gpsimd if tail else nc.sync
            eng.dma_start(wo_nf[:], wo_e[g, nf])
            return wo_nf

        out_acc = sb.tile([P, RQ, D], FP)

        def out_proj_m(g, nf, wo_nf, m):
            """Emit one [128-row, 512-col] tile of group g's out-projection.
            Groups 0-2 accumulate (descaled by 1/16) into bf16 SBUF; group 3
            folds the running accumulator back in with a 16*I matmul, then
            the finishing descale-copy alternates ACT/DVE to halve the tail."""
            oT = oT_tiles[g]
            ms = slice(m * P, (m + 1) * P)
            ps = pp.tile([P, 512], FP, tag="pp", name="ops")
            for ch in range(4):
                nc.tensor.matmul(ps[:], lhsT=oT[:, ch, ms],
                                 rhs=wo_nf[:, ch, :],
                                 start=(ch == 0), stop=(ch == 3))
            acc = out_acc[:, m, nf * 512:(nf + 1) * 512]
            if g == 0:
                nc.vector.tensor_scalar_mul(acc, ps[:], ODSC)
            else:
                nc.vector.affine_then_add(acc, ps[:], acc, ODSC, 0.0)
            if g == 3:
                eng = nc.sync if (nf * RQ + m) % 2 else nc.gpsimd
                eng.dma_start(
                    out_e[m * P:(m + 1) * P, nf * 512:(nf + 1) * 512], acc)

        wo3_tiles = {}
        for g in range(4):                    # 4 groups of 4 pairs
            oT_tiles[g] = otp.tile([P, RQ, SQ], BF, tag="oT", name=f"oT_{g}")
            for pi in range(4):               # pairs within group
                pair = g * 4 + pi
                wo_cur = [None]
                kc = pair % 4                 # kv chunk holding both kv heads
                kva, kvb = 2 * (pair % 4), 2 * (pair % 4) + 1

                # seq-major attn.V: per head one psum bank holding 4 q-block
                # accumulators [128 q, 64 v + 1 ones]; col 64 collects the
                # softmax denominator per q row.  Exactly one start=True per
                # bank (the hw zero-region is bank-granular); every other
                # accumulator rides the same lazy zero fill.
                poA = po.tile([P, RQ, VW], FP, tag="poA", name="poA")
                poB = po.tile([P, RQ, VW], FP, tag="poB", name="poB")
                eabs = {}
                for c in range(SC + 2):
                    if c < SC:
                        # scores for both heads of the pair into one 2-bank
                        # psum tile; one exp op covers A and B
                        psAB = psc.tile([P, 1024], FP, tag="psc", name="psAB")
                        nc.tensor.matmul(psAB[:, 0:SQ],
                                         lhsT=kT[0:64, kc, c * P:(c + 1) * P],
                                         rhs=qT[0:64, pair, :],
                                         start=True, stop=True,
                                         tile_position=(0, 0))
                        nc.tensor.matmul(psAB[:, SQ:2 * SQ],
                                         lhsT=kT[64:128, kc, c * P:(c + 1) * P],
                                         rhs=qT[64:128, pair, :],
                                         start=True, stop=True,
                                         tile_position=(64, 0))
                        eab = epool.tile([P, 2, SQ], BF, tag="exp", name="eab")
                        nc.scalar.activation(eab[:], psAB[:], EXPF, scale=EXP_SCALE)
                        eabs[c] = eab
                    if c >= 2:
                        cc_ = c - 2      # attn.V lags two chunks behind exp
                        eab = eabs.pop(cc_)
                        for h, po_t, kvh in ((0, poA, kva), (1, poB, kvb)):
                            for qb in range(RQ):
                                nc.tensor.matmul(
                                    po_t[:, qb, :],
                                    lhsT=eab[:, h, qb * P:(qb + 1) * P],
                                    rhs=v_aug[:, kvh, cc_, :],
                                    start=(cc_ == 0 and qb == 0),
                                    stop=(cc_ == SC - 1),
                                    skip_group_check=True)
                    # spread next-pair q-proj and prev-group out-proj through
                    # the chunk loop so the PE never bunches them at the
                    # pair boundary (ACT rides its 1-chunk buffer)
                    if c == 1 and pair < 14:
                        wq_tiles[pair + 2] = wq_load(pair + 2)
                    if c == 3 and g >= 1:
                        wo_cur[0] = wo_load(g - 1, pi)
                    if c == 5 and pair < 14:
                        qproj(pair + 2)
                    if c == 7 and g == 3 and pi >= 2:
                        wo3_tiles[pi - 2] = wo_load(3, pi - 2)
                    if c in (9, 11, 13, 15) and g >= 1:
                        out_proj_m(g - 1, pi, wo_cur[0], (c - 9) // 2)
                # normalize (per-q denominator is a per-partition scalar;
                # the ones-col held 1/16 so rbc = 16/den and o_n = 16*o)
                # then transpose [q, (h d)] -> [(h d), q] on the DMA xbar
                o_n = onp.tile([P, RQ, 2, HD], BF, tag="on", name="o_n")
                rbcA = npool.tile([P, RQ], FP, tag="rbc", name="rbcA")
                rbcB = npool.tile([P, RQ], FP, tag="rbc", name="rbcB")
                nc.vector.reciprocal(rbcA[:], poA[:, :, HD:HD + 1])
                nc.vector.reciprocal(rbcB[:], poB[:, :, HD:HD + 1])
                for h, po_t, rbc in ((0, poA, rbcA), (1, poB, rbcB)):
                    nc.vector.tensor_mul(
                        o_n[:, :, h, :], po_t[:, :, 0:HD],
                        rbc[:].rearrange("p q -> p q ()").broadcast_to(
                            (P, RQ, HD)))
                last_pair = pair == NQ // 2 - 1
                for qb in range(RQ):
                    # scalar queue only for the last pair (ACT is done with
                    # exps there; mid-kernel it would stall exp dispatch)
                    eng = nc.scalar if last_pair and qb % 2 else nc.sync
                    eng.dma_start_transpose(
                        oT_tiles[g][:, pi, qb * P:(qb + 1) * P],
                        o_n[:, qb, :, :])

            if g == 3:
                for nf in range(4):
                    if nf + 2 < 4:
                        wo3_tiles[nf + 2] = wo_load(3, nf + 2, tail=True)
                    for m in range(RQ):
                        out_proj_m(3, nf, wo3_tiles[nf], m)

    nc.compile()
    return nc


_NC = None


def _get_nc():
    global _NC
    if _NC is None:
        _NC = build()
    return _NC


def _split8(a, scale):
    """Scaled fp8 hi/lo split: a*scale == hi + lo to ~11 mantissa bits."""
    s = (a * scale).astype(np.float32)
    hi = s.astype(F8H)
    lo = (s - hi.astype(np.float32)).astype(F8H)
    return np.ascontiguousarray(hi), np.ascontiguousarray(lo)


def _host_prep(inputs):
    """Swizzle all weights into the on-chip layouts (so device DMAs are
    linear), pre-split everything into scaled fp8 hi/lo pairs, build the
    interleaved-layout CC/SS tables, slice per-core shards.  Q/K features
    keep their natural interleaved order (e0 o0 e1 o1 ...): the RoPE partner
    swap is then an adjacent-partition stream_shuffle on the device."""
    x = np.asarray(inputs["x"], np.float32)
    cos = np.asarray(inputs["cos"], np.float32)
    sin = np.asarray(inputs["sin"], np.float32)
    wq = np.asarray(inputs["wq"], np.float32)
    wk = np.asarray(inputs["wk"], np.float32)
    wv = np.asarray(inputs["wv"], np.float32)
    wo = np.asarray(inputs["wo"], np.float32)

    # device layouts (f32, split to fp8 at the end)
    wq_dev = np.ascontiguousarray(
        wq.reshape(DC, P, DC, P).transpose(2, 1, 0, 3))
    wk_dev = np.ascontiguousarray(
        wk.reshape(DC, P, KF).transpose(1, 0, 2))
    wv_dev = np.ascontiguousarray(
        wv.reshape(DC, P, KF).transpose(1, 0, 2))
    wo_dev = np.ascontiguousarray(
        wo.reshape(RQ, RQ, P, RQ, 512).transpose(0, 3, 2, 1, 4))

    wqh, wql = _split8(wq_dev, WS)
    wkh, wkl = _split8(wk_dev, WS)
    wvh, wvl = _split8(wv_dev, WS)
    wo_bf = np.ascontiguousarray(wo_dev.astype(ml_dtypes.bfloat16))

    cosT = np.ascontiguousarray(cos.T)            # [32, S]
    sinT = np.ascontiguousarray(sin.T)
    cos2 = np.repeat(cosT, 2, axis=0)             # [64, S] rows c0 c0 c1 c1..
    sin2 = np.repeat(sinT, 2, axis=0)
    sign = np.tile(np.array([-1.0, 1.0], np.float32), 32)[:, None]
    CC = np.tile(cos2, (2, 1)) * DSC              # [128, S]; fp8 descale
    SS = np.tile(sin2 * sign, (2, 1)) * DSC

    id8k = np.ascontiguousarray(
        np.eye(P, dtype=np.float32) * XS).astype(ml_dtypes.bfloat16)

    in_maps = []
    for c in range(NCORES):
        b, q = c // GPB, c % GPB
        sl = slice(q * SQ, (q + 1) * SQ)
        x_dev = np.ascontiguousarray(
            x[b, sl, :].T.reshape(DC, P, SQ).transpose(1, 0, 2))
        xh, xl = _split8(x_dev, XS)
        in_maps.append({
            "xh": xh, "xl": xl,
            "wqh": wqh, "wql": wql, "wkh": wkh, "wkl": wkl,
            "wvh": wvh, "wvl": wvl, "wo": wo_bf,
            "cc": np.ascontiguousarray(CC[:, sl]),
            "ss": np.ascontiguousarray(SS[:, sl]),
            "id8k": id8k,
        })
    return in_maps


def kernel(**inputs):
    nc = _get_nc()
    in_maps = _host_prep(inputs)
    res = run_bass_kernel_spmd(nc, in_maps, core_ids=list(range(NCORES)))
    out = np.empty((B, S, D), np.float32)
    for c in range(NCORES):
        b, q = c // GPB, c % GPB
        out[b, q * SQ:(q + 1) * SQ, :] = res.results[c]["out"]
    return out


# revision 74
# speedup vs baseline: 1.0356x; 1.0006x over previous
"""Distributed GQA attention (llama-style RoPE) for one TRN2 chip (8 NeuronCores).

Sharding: core c handles batch b=c//4 and sequence-quarter q=c%4 (512 q-rows).
Each core projects Q for its own rows (all 32 heads), projects K/V for its own
512 positions, AllGathers K/V within its 4-core batch group, runs attention for
its rows, and applies the output projection. Output rows are disjoint across
cores, so no all-reduce is needed; the host concatenates.

On-chip dataflow (per core):
  xT    = x.T, shipped from host pre-split into fp8 hi+lo (x16 scale)
  Q/K/V/out projections run as compensated-fp8 DoubleRow matmuls:
  3 passes (hi*hi + lo*hi + hi*lo) of half-cost fp8 matmuls == 0.75x the
  bf16 cycle count at slightly BETTER-than-bf16 accuracy.  Weight scale
  (x512) and activation scale (x16) are divided back out in the psum
  consumers (RoPE tables, v copy, the softmax ones-column, out accumulate).
  per head pair (with the previous group's out-proj interleaved):
    qT   = wq.T @ xT -> RoPE -> bf16                  [feat, sq]
    sT   = kT_h.T @ qT_h  (row-packed pairs)          [skv, sq]  psum f32
    e    = exp(sT/8) on ScalarE -> bf16
    o    = eab.T @ [v|1/16] seq-major, accumulated    [sq=4x128, 65] psum
           (col 64 collects the softmax denominator per q row; one
            start=True per psum bank, hw zero-region is bank-granular)
    o_n  = o * (16/den)  (per-partition scalar)  -> bf16 (=16*o)
    oT   = dma-xbar transpose of o_n -> [feat, sq]
  out  = oT.T @ wo (bf16), 4 head-group partials summed in f32 SBUF via
         tensor_scalar / affine_then_add with the 1/16 descale folded in.

All weights are pre-swizzled on the HOST into the exact SBUF layouts so every
DMA is a fully-linear copy.  Q/K features stay in natural interleaved order
(e0 o0 e1 o1 ...): the RoPE partner swap is an adjacent-partition
stream_shuffle on the DVE, and the cos/sin tables are shipped row-duplicated
and sign-baked as CC/SS [128, sq] with the fp8 descale folded in.
"""
import sys

sys.path.insert(0, "/opt/trn_rl_repo")

import numpy as np
import ml_dtypes
from contextlib import ExitStack

import concourse.bass as bass
import concourse.mybir as mybir
import concourse.tile as tile
from concourse import bacc
from concourse.bass_utils import run_bass_kernel_spmd
from concourse.masks import make_identity

B, S, D = 2, 2048, 2048
NQ, NKV, HD = 32, 8, 64
NCORES = 8
GPB = 4                 # cores per batch group
SQ = S // GPB           # 512 q-rows per core
P = 128
DC = D // P             # 16 contraction chunks
KF = NKV * HD           # 512 kv feature dim
KFC = KF // P           # 4 kv feature chunks
SC = S // P             # 16 skv chunks
RQ = SQ // P            # 4 q-row blocks
VW = HD + 1             # v_aug width: 64 v cols + 1 ones col (denominator)

FP = mybir.dt.float32
BF = mybir.dt.bfloat16
F8 = mybir.dt.float8e4
F8H = ml_dtypes.float8_e4m3
DR = mybir.MatmulPerfMode.DoubleRow
EXPF = mybir.ActivationFunctionType.Exp
COPYF = mybir.ActivationFunctionType.Copy
EXP_SCALE = 1.0 / 8.0   # 1/sqrt(HD)

XS = 16.0               # fp8 pre-scale on activations
WS = 512.0              # fp8 pre-scale on weights
DSC = 1.0 / (XS * WS)   # descale folded into psum consumers
ODSC = 1.0 / XS         # out-proj descale (oT carries x16; wo is bf16)


def build(solo=False):
    nc = bacc.Bacc("TRN2", target_bir_lowering=False, debug=False,
                   num_devices=1 if solo else NCORES)

    xh_e = nc.dram_tensor("xh", [P, DC, SQ], F8, kind="ExternalInput").ap()
    xl_e = nc.dram_tensor("xl", [P, DC, SQ], F8, kind="ExternalInput").ap()
    wqh_e = nc.dram_tensor("wqh", [DC, P, DC, P], F8, kind="ExternalInput").ap()
    wql_e = nc.dram_tensor("wql", [DC, P, DC, P], F8, kind="ExternalInput").ap()
    wkh_e = nc.dram_tensor("wkh", [P, DC, KF], F8, kind="ExternalInput").ap()
    wkl_e = nc.dram_tensor("wkl", [P, DC, KF], F8, kind="ExternalInput").ap()
    wvh_e = nc.dram_tensor("wvh", [P, DC, KF], F8, kind="ExternalInput").ap()
    wvl_e = nc.dram_tensor("wvl", [P, DC, KF], F8, kind="ExternalInput").ap()
    wo_e = nc.dram_tensor("wo", [RQ, RQ, P, RQ, 512], BF, kind="ExternalInput").ap()
    cc_e = nc.dram_tensor("cc", [P, SQ], FP, kind="ExternalInput").ap()
    ss_e = nc.dram_tensor("ss", [P, SQ], FP, kind="ExternalInput").ap()
    id8k_e = nc.dram_tensor("id8k", [P, P], BF, kind="ExternalInput").ap()
    out_e = nc.dram_tensor("out", [SQ, D], FP, kind="ExternalOutput").ap()

    groups = [[0, 1, 2, 3], [4, 5, 6, 7]]

    with tile.TileContext(nc) as tc, ExitStack() as ctx:
        sb = ctx.enter_context(tc.tile_pool(name="sb", bufs=1))
        rp = ctx.enter_context(tc.tile_pool(name="rp", bufs=3))
        epool = ctx.enter_context(tc.tile_pool(name="epool", bufs=6))
        npool = ctx.enter_context(tc.tile_pool(name="npool", bufs=3))
        onp = ctx.enter_context(tc.tile_pool(name="onp", bufs=3))
        opool = ctx.enter_context(tc.tile_pool(name="opool", bufs=3))
        otp = ctx.enter_context(tc.tile_pool(name="otp", bufs=2))
        early = ctx.enter_context(tc.tile_pool(name="early", bufs=1))
        wqp = ctx.enter_context(tc.tile_pool(name="wqp", bufs=3))
        dram = ctx.enter_context(tc.tile_pool(name="dram", bufs=1, space="DRAM"))
        pp = ctx.enter_context(tc.tile_pool(name="pp", bufs=2, space="PSUM"))
        psc = ctx.enter_context(tc.tile_pool(name="psc", bufs=2, space="PSUM"))
        po = ctx.enter_context(tc.tile_pool(name="po", bufs=1, space="PSUM"))

        # ---- constants ----
        cc_sb = sb.tile([P, SQ], FP)
        ss_sb = sb.tile([P, SQ], FP)
        id8k = sb.tile([P, P], BF)      # 8192 * identity (descale-matched)

        # features stay in natural interleaved order (e0 o0 e1 o1 ...): the
        # RoPE partner swap is adjacent-partition, expressible as an intra-
        # quadrant stream_shuffle; score contraction is order-invariant.
        SWAP_MASK = [i ^ 1 for i in range(32)]

        def rope_chunk(ps, dst):
            """dst = RoPE(ps)*DSC in interleaved layout; ps [128,SQ] psum."""
            t0 = rp.tile([P, SQ], FP, tag="t0")
            tsh = rp.tile([P, SQ], FP, tag="tsh")
            t1 = rp.tile([P, SQ], FP, tag="t1")
            nc.vector.tensor_mul(t0[:], ps[:], cc_sb[:])
            nc.vector.stream_shuffle(tsh[:], ps[:], SWAP_MASK)
            nc.vector.tensor_mul(t1[:], tsh[:], ss_sb[:])
            nc.vector.tensor_add(dst, t0[:], t1[:])

        qT = sb.tile([P, DC, SQ], BF)
        kag_in = dram.tile([KF, SQ], BF)
        kag_out = dram.tile([GPB * KF, SQ], BF)
        vag_in = dram.tile([SQ, KF], BF)
        vag_out = dram.tile([S, KF], BF)

        # ---- pure input loads first, split across queues: wq + x on the
        #      sync queue, the K/V/id weights on the (early-idle) scalar
        #      queue, wo on the gpsimd swdge queue later ----
        wq_tiles = {}

        def wq_load(pair):
            wh = wqp.tile([P, DC, P], F8, tag="wqh", name=f"wqh_{pair}")
            wl = wqp.tile([P, DC, P], F8, tag="wql", name=f"wql_{pair}")
            nc.sync.dma_start(wh[:], wqh_e[pair])
            nc.sync.dma_start(wl[:], wql_e[pair])
            return wh, wl

        # one queue, strict consumption order: the DMA pipe is a single
        # serialized resource in practice, so emission order = arrival order.
        # K comes FIRST so the AllGather (the longest dependency chain of
        # the attention phase) is in flight as early as possible.
        xTh = early.tile([P, DC, SQ], F8, tag="xTh", name="xTh")
        xTl = early.tile([P, DC, SQ], F8, tag="xTl", name="xTl")
        wkh = early.tile([P, DC, KF], F8, tag="wkh", name="wkh")
        wkl = early.tile([P, DC, KF], F8, tag="wkl", name="wkl")
        wvh = early.tile([P, DC, KF], F8, tag="wvh", name="wvh")
        wvl = early.tile([P, DC, KF], F8, tag="wvl", name="wvl")

        def x_load(xc):
            s4 = slice(4 * xc, 4 * (xc + 1))
            nc.sync.dma_start(xTh[:, s4, :], xh_e[:, s4, :])
            nc.sync.dma_start(xTl[:, s4, :], xl_e[:, s4, :])

        nc.sync.dma_start(wkh[:], wkh_e)
        for xc in range(4):
            s4 = slice(4 * xc, 4 * (xc + 1))
            nc.sync.dma_start(xTh[:, s4, :], xh_e[:, s4, :])
        nc.sync.dma_start(wkl[:], wkl_e)
        for xc in range(4):
            s4 = slice(4 * xc, 4 * (xc + 1))
            nc.sync.dma_start(xTl[:, s4, :], xl_e[:, s4, :])
        nc.sync.dma_start(cc_sb[:], cc_e)
        nc.sync.dma_start(ss_sb[:], ss_e)
        wq_tiles[0] = wq_load(0)
        nc.sync.dma_start(wvh[:], wvh_e)
        nc.sync.dma_start(wvl[:], wvl_e)
        wq_tiles[1] = wq_load(1)
        nc.sync.dma_start(id8k[:], id8k_e)

        def comp_passes():
            """(x, w, first) triples for the 3 compensated DR passes."""
            return ((xTh, 0, True), (xTl, 0, False), (xTh, 1, False))

        def qproj(pair):
            wh, wl = wq_tiles.pop(pair)
            qps = pp.tile([P, 512], FP, tag="pp", name="qps")
            for i in range(DC // 2):
                s2 = slice(2 * i, 2 * i + 2)
                for xs, wlo, first in comp_passes():
                    w = wl if wlo else wh
                    nc.tensor.matmul(qps[:, :SQ], lhsT=w[:, s2, :],
                                     rhs=xs[:, s2, :],
                                     start=(first and i == 0),
                                     stop=(i == DC // 2 - 1 and wlo == 1),
                                     perf_mode=DR)
            rope_chunk(qps[:, :SQ], qT[:, pair, :])

        # ---- K projection + RoPE -> AllGather (before everything else) ----
        kT_own = sb.tile([P, KFC, SQ], BF, tag="own4", name="kT_own")
        for fc in range(KFC):
            ps = pp.tile([P, 512], FP, tag="pp", name="kps")
            fs = slice(fc * P, (fc + 1) * P)
            # pass-major: the first pass needs only wkh + the xh chunks, so
            # the PE starts before wkl/xl even arrive
            for xs, w, first, last in ((xTh, wkh, True, False),
                                       (xTh, wkl, False, False),
                                       (xTl, wkh, False, True)):
                for i in range(DC // 2):
                    s2 = slice(2 * i, 2 * i + 2)
                    nc.tensor.matmul(ps[:, :SQ], lhsT=w[:, s2, fs],
                                     rhs=xs[:, s2, :],
                                     start=(first and i == 0),
                                     stop=(last and i == DC // 2 - 1),
                                     perf_mode=DR)
            rope_chunk(ps[:, :SQ], kT_own[:, fc, :])
        nc.scalar.dma_start(kag_in[:].rearrange("(c p) s -> p c s", p=P),
                            kT_own[:])
        if solo:
            for r in range(GPB):
                nc.scalar.dma_start(kag_out[r * KF:(r + 1) * KF, :], kag_in[:])
        else:
            nc.gpsimd.collective_compute(
                "AllGather", mybir.AluOpType.bypass, replica_groups=groups,
                ins=[kag_in[:]], outs=[kag_out[:]])

        # ---- land gathered K (kT reuses the wk slot) while Q0 projects ----
        kT = early.tile([P, KFC, S], BF, tag="wkh", name="kT")
        kag_v = kag_out[:].rearrange("(r c p) s -> c p r s", r=GPB, p=P)
        for fc in range(KFC):
            nc.scalar.dma_start(
                kT[:, fc, :].rearrange("p (r s) -> p r s", r=GPB), kag_v[fc])
        qproj(0)

        # ---- V projection -> AllGather (x is the stationary side) ----
        v_own = sb.tile([P, RQ, KF], BF, tag="own4", name="v_own")
        for pc in range(RQ):
            ps = pp.tile([P, 512], FP, tag="pp", name="vps")
            pcs = slice(pc * P, (pc + 1) * P)
            for i in range(DC // 2):
                s2 = slice(2 * i, 2 * i + 2)
                for xs, wlo, first in comp_passes():
                    w = wvl if wlo else wvh
                    nc.tensor.matmul(ps[:, :KF], lhsT=xs[:, s2, pcs],
                                     rhs=w[:, s2, :],
                                     start=(first and i == 0),
                                     stop=(i == DC // 2 - 1 and wlo == 1),
                                     perf_mode=DR)
            nc.vector.tensor_scalar_mul(v_own[:, pc, :], ps[:, :KF], DSC)
            nc.sync.dma_start(vag_in[pc * P:(pc + 1) * P, :], v_own[:, pc, :])
        if solo:
            for r in range(GPB):
                nc.sync.dma_start(vag_out[r * SQ:(r + 1) * SQ, :], vag_in[:])
        else:
            nc.gpsimd.collective_compute(
                "AllGather", mybir.AluOpType.bypass, replica_groups=groups,
                ins=[vag_in[:]], outs=[vag_out[:]])

        qproj(1)

        v_aug = early.tile([P, NKV, SC, VW], BF, tag="wvh", name="v_aug")
        # only the ones-col needs the memset; the DMAs below fill cols 0:HD
        nc.gpsimd.memset(v_aug[:, :, :, HD:HD + 1], 1.0 / XS)
        for c in range(SC):
            nc.sync.dma_start(
                v_aug[:, :, c, 0:HD],
                vag_out[c * P:(c + 1) * P, :].rearrange("p (kv d) -> p kv d", d=HD))

        # ---- per-pair: Q proj + attention; prev group's out-proj interleaved ----
        oT_tiles = {}

        def wo_load(g, nf, tail=False):
            wo_nf = opool.tile([P, 4, 512], BF, tag="wo", name="wo_nf")
            # tail loads go on the swdge queue: the sync queue head-of-line
            # blocks on the last pair's transposes right then
            eng = nc.gpsimd if tail else nc.sync
            eng.dma_start(wo_nf[:], wo_e[g, nf])
            return wo_nf

        out_acc = sb.tile([P, RQ, D], FP)

        def out_proj_m(g, nf, wo_nf, m):
            """Emit one [128-row, 512-col] tile of group g's out-projection.
            Groups 0-2 accumulate (descaled by 1/16) into bf16 SBUF; group 3
            folds the running accumulator back in with a 16*I matmul, then
            the finishing descale-copy alternates ACT/DVE to halve the tail."""
            oT = oT_tiles[g]
            ms = slice(m * P, (m + 1) * P)
            ps = pp.tile([P, 512], FP, tag="pp", name="ops")
            for ch in range(4):
                nc.tensor.matmul(ps[:], lhsT=oT[:, ch, ms],
                                 rhs=wo_nf[:, ch, :],
                                 start=(ch == 0), stop=(ch == 3))
            acc = out_acc[:, m, nf * 512:(nf + 1) * 512]
            if g == 0:
                nc.vector.tensor_scalar_mul(acc, ps[:], ODSC)
            else:
                nc.vector.affine_then_add(acc, ps[:], acc, ODSC, 0.0)
            if g == 3:
                eng = nc.sync if (nf * RQ + m) % 2 else nc.gpsimd
                eng.dma_start(
                    out_e[m * P:(m + 1) * P, nf * 512:(nf + 1) * 512], acc)

        wo3_tiles = {}
        for g in range(4):                    # 4 groups of 4 pairs
            oT_tiles[g] = otp.tile([P, RQ, SQ], BF, tag="oT", name=f"oT_{g}")
            for pi in range(4):               # pairs within group
                pair = g * 4 + pi
                wo_cur = [None]
                kc = pair % 4                 # kv chunk holding both kv heads
                kva, kvb = 2 * (pair % 4), 2 * (pair % 4) + 1

                # seq-major attn.V: per head one psum bank holding 4 q-block
                # accumulators [128 q, 64 v + 1 ones]; col 64 collects the
                # softmax denominator per q row.  Exactly one start=True per
                # bank (the hw zero-region is bank-granular); every other
                # accumulator rides the same lazy zero fill.
                poA = po.tile([P, RQ, VW], FP, tag="poA", name="poA")
                poB = po.tile([P, RQ, VW], FP, tag="poB", name="poB")
                eabs = {}
                for c in range(SC + 3):
                    if c < SC:
                        # scores for both heads of the pair into one 2-bank
                        # psum tile; one exp op covers A and B
                        psAB = psc.tile([P, 1024], FP, tag="psc", name="psAB")
                        nc.tensor.matmul(psAB[:, 0:SQ],
                                         lhsT=kT[0:64, kc, c * P:(c + 1) * P],
                                         rhs=qT[0:64, pair, :],
                                         start=True, stop=True,
                                         tile_position=(0, 0))
                        nc.tensor.matmul(psAB[:, SQ:2 * SQ],
                                         lhsT=kT[64:128, kc, c * P:(c + 1) * P],
                                         rhs=qT[64:128, pair, :],
                                         start=True, stop=True,
                                         tile_position=(64, 0))
                        eab = epool.tile([P, 2, SQ], BF, tag="exp", name="eab")
                        nc.scalar.activation(eab[:], psAB[:], EXPF, scale=EXP_SCALE)
                        eabs[c] = eab
                    if c >= 3:
                        cc_ = c - 3      # attn.V lags three chunks behind exp
                        eab = eabs.pop(cc_)
                        for h, po_t, kvh in ((0, poA, kva), (1, poB, kvb)):
                            for qb in range(RQ):
                                nc.tensor.matmul(
                                    po_t[:, qb, :],
                                    lhsT=eab[:, h, qb * P:(qb + 1) * P],
                                    rhs=v_aug[:, kvh, cc_, :],
                                    start=(cc_ == 0 and qb == 0),
                                    stop=(cc_ == SC - 1),
                                    skip_group_check=True)
                    # spread next-pair q-proj and prev-group out-proj through
                    # the chunk loop so the PE never bunches them at the
                    # pair boundary (ACT rides its 1-chunk buffer)
                    if c == 1 and pair < 14:
                        wq_tiles[pair + 2] = wq_load(pair + 2)
                    if c == 3 and g >= 1:
                        wo_cur[0] = wo_load(g - 1, pi)
                    if c == 5 and pair < 14:
                        qproj(pair + 2)
                    if c == 7 and g == 3 and pi >= 2:
                        wo3_tiles[pi - 2] = wo_load(3, pi - 2)
                    if c in (9, 11, 13, 15) and g >= 1:
                        out_proj_m(g - 1, pi, wo_cur[0], (c - 9) // 2)
                # normalize (per-q denominator is a per-partition scalar;
                # the ones-col held 1/16 so rbc = 16/den and o_n = 16*o)
                # then transpose [q, (h d)] -> [(h d), q] on the DMA xbar
                o_n = onp.tile([P, RQ, 2, HD], BF, tag="on", name="o_n")
                rbcA = npool.tile([P, RQ], FP, tag="rbc", name="rbcA")
                rbcB = npool.tile([P, RQ], FP, tag="rbc", name="rbcB")
                nc.vector.reciprocal(rbcA[:], poA[:, :, HD:HD + 1])
                nc.vector.reciprocal(rbcB[:], poB[:, :, HD:HD + 1])
                for h, po_t, rbc in ((0, poA, rbcA), (1, poB, rbcB)):
                    nc.vector.tensor_mul(
                        o_n[:, :, h, :], po_t[:, :, 0:HD],
                        rbc[:].rearrange("p q -> p q ()").broadcast_to(
                            (P, RQ, HD)))
                last_pair = pair == NQ // 2 - 1
                for qb in range(RQ):
                    # scalar queue only for the last pair (ACT is done with
                    # exps there; mid-kernel it would stall exp dispatch)
                    eng = nc.scalar if last_pair and qb % 2 else nc.sync
                    eng.dma_start_transpose(
                        oT_tiles[g][:, pi, qb * P:(qb + 1) * P],
                        o_n[:, qb, :, :])

            if g == 3:
                for nf in range(4):
                    if nf + 2 < 4:
                        wo3_tiles[nf + 2] = wo_load(3, nf + 2, tail=True)
                    for m in range(RQ):
                        out_proj_m(3, nf, wo3_tiles[nf], m)

    nc.compile()
    return nc


_NC = None


def _get_nc():
    global _NC
    if _NC is None:
        _NC = build()
    return _NC


def _split8(a, scale):
    """Scaled fp8 hi/lo split: a*scale == hi + lo to ~11 mantissa bits."""
    s = (a * scale).astype(np.float32)
    hi = s.astype(F8H)
    lo = (s - hi.astype(np.float32)).astype(F8H)
    return np.ascontiguousarray(hi), np.ascontiguousarray(lo)


def _host_prep(inputs):
    """Swizzle all weights into the on-chip layouts (so device DMAs are
    linear), pre-split everything into scaled fp8 hi/lo pairs, build the
    interleaved-layout CC/SS tables, slice per-core shards.  Q/K features
    keep their natural interleaved order (e0 o0 e1 o1 ...): the RoPE partner
    swap is then an adjacent-partition stream_shuffle on the device."""
    x = np.asarray(inputs["x"], np.float32)
    cos = np.asarray(inputs["cos"], np.float32)
    sin = np.asarray(inputs["sin"], np.float32)
    wq = np.asarray(inputs["wq"], np.float32)
    wk = np.asarray(inputs["wk"], np.float32)
    wv = np.asarray(inputs["wv"], np.float32)
    wo = np.asarray(inputs["wo"], np.float32)

    # device layouts (f32, split to fp8 at the end)
    wq_dev = np.ascontiguousarray(
        wq.reshape(DC, P, DC, P).transpose(2, 1, 0, 3))
    wk_dev = np.ascontiguousarray(
        wk.reshape(DC, P, KF).transpose(1, 0, 2))
    wv_dev = np.ascontiguousarray(
        wv.reshape(DC, P, KF).transpose(1, 0, 2))
    wo_dev = np.ascontiguousarray(
        wo.reshape(RQ, RQ, P, RQ, 512).transpose(0, 3, 2, 1, 4))

    wqh, wql = _split8(wq_dev, WS)
    wkh, wkl = _split8(wk_dev, WS)
    wvh, wvl = _split8(wv_dev, WS)
    wo_bf = np.ascontiguousarray(wo_dev.astype(ml_dtypes.bfloat16))

    cosT = np.ascontiguousarray(cos.T)            # [32, S]
    sinT = np.ascontiguousarray(sin.T)
    cos2 = np.repeat(cosT, 2, axis=0)             # [64, S] rows c0 c0 c1 c1..
    sin2 = np.repeat(sinT, 2, axis=0)
    sign = np.tile(np.array([-1.0, 1.0], np.float32), 32)[:, None]
    CC = np.tile(cos2, (2, 1)) * DSC              # [128, S]; fp8 descale
    SS = np.tile(sin2 * sign, (2, 1)) * DSC

    id8k = np.ascontiguousarray(
        np.eye(P, dtype=np.float32) * XS).astype(ml_dtypes.bfloat16)

    in_maps = []
    for c in range(NCORES):
        b, q = c // GPB, c % GPB
        sl = slice(q * SQ, (q + 1) * SQ)
        x_dev = np.ascontiguousarray(
            x[b, sl, :].T.reshape(DC, P, SQ).transpose(1, 0, 2))
        xh, xl = _split8(x_dev, XS)
        in_maps.append({
            "xh": xh, "xl": xl,
            "wqh": wqh, "wql": wql, "wkh": wkh, "wkl": wkl,
            "wvh": wvh, "wvl": wvl, "wo": wo_bf,
            "cc": np.ascontiguousarray(CC[:, sl]),
            "ss": np.ascontiguousarray(SS[:, sl]),
            "id8k": id8k,
        })
    return in_maps


def kernel(**inputs):
    nc = _get_nc()
    in_maps = _host_prep(inputs)
    res = run_bass_kernel_spmd(nc, in_maps, core_ids=list(range(NCORES)))
    out = np.empty((B, S, D), np.float32)
    for c in range(NCORES):
        b, q = c // GPB, c % GPB
        out[b, q * SQ:(q + 1) * SQ, :] = res.results[c]["out"]
    return out


# revision 75
# speedup vs baseline: 1.0357x; 1.0001x over previous
"""Distributed GQA attention (llama-style RoPE) for one TRN2 chip (8 NeuronCores).

Sharding: core c handles batch b=c//4 and sequence-quarter q=c%4 (512 q-rows).
Each core projects Q for its own rows (all 32 heads), projects K/V for its own
512 positions, AllGathers K/V within its 4-core batch group, runs attention for
its rows, and applies the output projection. Output rows are disjoint across
cores, so no all-reduce is needed; the host concatenates.

On-chip dataflow (per core):
  xT    = x.T, shipped from host pre-split into fp8 hi+lo (x16 scale)
  Q/K/V/out projections run as compensated-fp8 DoubleRow matmuls:
  3 passes (hi*hi + lo*hi + hi*lo) of half-cost fp8 matmuls == 0.75x the
  bf16 cycle count at slightly BETTER-than-bf16 accuracy.  Weight scale
  (x512) and activation scale (x16) are divided back out in the psum
  consumers (RoPE tables, v copy, the softmax ones-column, out accumulate).
  per head pair (with the previous group's out-proj interleaved):
    qT   = wq.T @ xT -> RoPE -> bf16                  [feat, sq]
    sT   = kT_h.T @ qT_h  (row-packed pairs)          [skv, sq]  psum f32
    e    = exp(sT/8) on ScalarE -> bf16
    o    = eab.T @ [v|1/16] seq-major, accumulated    [sq=4x128, 65] psum
           (col 64 collects the softmax denominator per q row; one
            start=True per psum bank, hw zero-region is bank-granular)
    o_n  = o * (16/den)  (per-partition scalar)  -> bf16 (=16*o)
    oT   = dma-xbar transpose of o_n -> [feat, sq]
  out  = oT.T @ wo (bf16), 4 head-group partials summed in f32 SBUF via
         tensor_scalar / affine_then_add with the 1/16 descale folded in.

All weights are pre-swizzled on the HOST into the exact SBUF layouts so every
DMA is a fully-linear copy.  Q/K features stay in natural interleaved order
(e0 o0 e1 o1 ...): the RoPE partner swap is an adjacent-partition
stream_shuffle on the DVE, and the cos/sin tables are shipped row-duplicated
and sign-baked as CC/SS [128, sq] with the fp8 descale folded in.
"""
import sys

sys.path.insert(0, "/opt/trn_rl_repo")

import numpy as np
import ml_dtypes
from contextlib import ExitStack

import concourse.bass as bass
import concourse.mybir as mybir
import concourse.tile as tile
from concourse import bacc
from concourse.bass_utils import run_bass_kernel_spmd
from concourse.masks import make_identity

B, S, D = 2, 2048, 2048
NQ, NKV, HD = 32, 8, 64
NCORES = 8
GPB = 4                 # cores per batch group
SQ = S // GPB           # 512 q-rows per core
P = 128
DC = D // P             # 16 contraction chunks
KF = NKV * HD           # 512 kv feature dim
KFC = KF // P           # 4 kv feature chunks
SC = S // P             # 16 skv chunks
RQ = SQ // P            # 4 q-row blocks
VW = HD + 1             # v_aug width: 64 v cols + 1 ones col (denominator)

FP = mybir.dt.float32
BF = mybir.dt.bfloat16
F8 = mybir.dt.float8e4
F8H = ml_dtypes.float8_e4m3
DR = mybir.MatmulPerfMode.DoubleRow
EXPF = mybir.ActivationFunctionType.Exp
COPYF = mybir.ActivationFunctionType.Copy
EXP_SCALE = 1.0 / 8.0   # 1/sqrt(HD)

XS = 16.0               # fp8 pre-scale on activations
WS = 512.0              # fp8 pre-scale on weights
DSC = 1.0 / (XS * WS)   # descale folded into psum consumers
ODSC = 1.0 / XS         # out-proj descale (oT carries x16; wo is bf16)


def build(solo=False):
    nc = bacc.Bacc("TRN2", target_bir_lowering=False, debug=False,
                   num_devices=1 if solo else NCORES)

    xh_e = nc.dram_tensor("xh", [P, DC, SQ], F8, kind="ExternalInput").ap()
    xl_e = nc.dram_tensor("xl", [P, DC, SQ], F8, kind="ExternalInput").ap()
    wqh_e = nc.dram_tensor("wqh", [DC, P, DC, P], F8, kind="ExternalInput").ap()
    wql_e = nc.dram_tensor("wql", [DC, P, DC, P], F8, kind="ExternalInput").ap()
    wkh_e = nc.dram_tensor("wkh", [P, DC, KF], F8, kind="ExternalInput").ap()
    wkl_e = nc.dram_tensor("wkl", [P, DC, KF], F8, kind="ExternalInput").ap()
    wvh_e = nc.dram_tensor("wvh", [P, DC, KF], F8, kind="ExternalInput").ap()
    wvl_e = nc.dram_tensor("wvl", [P, DC, KF], F8, kind="ExternalInput").ap()
    wo_e = nc.dram_tensor("wo", [RQ, RQ, P, RQ, 512], BF, kind="ExternalInput").ap()
    cc_e = nc.dram_tensor("cc", [P, SQ], FP, kind="ExternalInput").ap()
    ss_e = nc.dram_tensor("ss", [P, SQ], FP, kind="ExternalInput").ap()
    id8k_e = nc.dram_tensor("id8k", [P, P], BF, kind="ExternalInput").ap()
    out_e = nc.dram_tensor("out", [SQ, D], FP, kind="ExternalOutput").ap()

    groups = [[0, 1, 2, 3], [4, 5, 6, 7]]

    with tile.TileContext(nc) as tc, ExitStack() as ctx:
        sb = ctx.enter_context(tc.tile_pool(name="sb", bufs=1))
        rp = ctx.enter_context(tc.tile_pool(name="rp", bufs=3))
        epool = ctx.enter_context(tc.tile_pool(name="epool", bufs=6))
        npool = ctx.enter_context(tc.tile_pool(name="npool", bufs=3))
        onp = ctx.enter_context(tc.tile_pool(name="onp", bufs=3))
        opool = ctx.enter_context(tc.tile_pool(name="opool", bufs=3))
        otp = ctx.enter_context(tc.tile_pool(name="otp", bufs=2))
        early = ctx.enter_context(tc.tile_pool(name="early", bufs=1))
        wqp = ctx.enter_context(tc.tile_pool(name="wqp", bufs=3))
        dram = ctx.enter_context(tc.tile_pool(name="dram", bufs=1, space="DRAM"))
        pp = ctx.enter_context(tc.tile_pool(name="pp", bufs=2, space="PSUM"))
        psc = ctx.enter_context(tc.tile_pool(name="psc", bufs=2, space="PSUM"))
        po = ctx.enter_context(tc.tile_pool(name="po", bufs=1, space="PSUM"))

        # ---- constants ----
        cc_sb = sb.tile([P, SQ], FP)
        ss_sb = sb.tile([P, SQ], FP)
        id8k = sb.tile([P, P], BF)      # 8192 * identity (descale-matched)

        # features stay in natural interleaved order (e0 o0 e1 o1 ...): the
        # RoPE partner swap is adjacent-partition, expressible as an intra-
        # quadrant stream_shuffle; score contraction is order-invariant.
        SWAP_MASK = [i ^ 1 for i in range(32)]

        def rope_chunk(ps, dst):
            """dst = RoPE(ps)*DSC in interleaved layout; ps [128,SQ] psum."""
            t0 = rp.tile([P, SQ], FP, tag="t0")
            tsh = rp.tile([P, SQ], FP, tag="tsh")
            t1 = rp.tile([P, SQ], FP, tag="t1")
            nc.vector.tensor_mul(t0[:], ps[:], cc_sb[:])
            nc.vector.stream_shuffle(tsh[:], ps[:], SWAP_MASK)
            nc.vector.tensor_mul(t1[:], tsh[:], ss_sb[:])
            nc.vector.tensor_add(dst, t0[:], t1[:])

        qT = sb.tile([P, DC, SQ], BF)
        kag_in = dram.tile([KF, SQ], BF)
        kag_out = dram.tile([GPB * KF, SQ], BF)
        vag_in = dram.tile([SQ, KF], BF)
        vag_out = dram.tile([S, KF], BF)

        # ---- pure input loads first, split across queues: wq + x on the
        #      sync queue, the K/V/id weights on the (early-idle) scalar
        #      queue, wo on the gpsimd swdge queue later ----
        wq_tiles = {}

        def wq_load(pair):
            wh = wqp.tile([P, DC, P], F8, tag="wqh", name=f"wqh_{pair}")
            wl = wqp.tile([P, DC, P], F8, tag="wql", name=f"wql_{pair}")
            nc.sync.dma_start(wh[:], wqh_e[pair])
            nc.sync.dma_start(wl[:], wql_e[pair])
            return wh, wl

        # one queue, strict consumption order: the DMA pipe is a single
        # serialized resource in practice, so emission order = arrival order.
        # K comes FIRST so the AllGather (the longest dependency chain of
        # the attention phase) is in flight as early as possible.
        xTh = early.tile([P, DC, SQ], F8, tag="xTh", name="xTh")
        xTl = early.tile([P, DC, SQ], F8, tag="xTl", name="xTl")
        wkh = early.tile([P, DC, KF], F8, tag="wkh", name="wkh")
        wkl = early.tile([P, DC, KF], F8, tag="wkl", name="wkl")
        wvh = early.tile([P, DC, KF], F8, tag="wvh", name="wvh")
        wvl = early.tile([P, DC, KF], F8, tag="wvl", name="wvl")

        def x_load(xc):
            s4 = slice(4 * xc, 4 * (xc + 1))
            nc.sync.dma_start(xTh[:, s4, :], xh_e[:, s4, :])
            nc.sync.dma_start(xTl[:, s4, :], xl_e[:, s4, :])

        nc.sync.dma_start(wkh[:, 0:8, :], wkh_e[:, 0:8, :])
        nc.sync.dma_start(xTh[:, 0:4, :], xh_e[:, 0:4, :])
        nc.sync.dma_start(wkh[:, 8:16, :], wkh_e[:, 8:16, :])
        for xc in range(1, 4):
            s4 = slice(4 * xc, 4 * (xc + 1))
            nc.sync.dma_start(xTh[:, s4, :], xh_e[:, s4, :])
        nc.sync.dma_start(wkl[:], wkl_e)
        for xc in range(4):
            s4 = slice(4 * xc, 4 * (xc + 1))
            nc.sync.dma_start(xTl[:, s4, :], xl_e[:, s4, :])
        nc.sync.dma_start(cc_sb[:], cc_e)
        nc.sync.dma_start(ss_sb[:], ss_e)
        wq_tiles[0] = wq_load(0)
        nc.sync.dma_start(wvh[:], wvh_e)
        nc.sync.dma_start(wvl[:], wvl_e)
        wq_tiles[1] = wq_load(1)
        nc.sync.dma_start(id8k[:], id8k_e)

        def comp_passes():
            """(x, w, first) triples for the 3 compensated DR passes."""
            return ((xTh, 0, True), (xTl, 0, False), (xTh, 1, False))

        def qproj(pair):
            wh, wl = wq_tiles.pop(pair)
            qps = pp.tile([P, 512], FP, tag="pp", name="qps")
            for i in range(DC // 2):
                s2 = slice(2 * i, 2 * i + 2)
                for xs, wlo, first in comp_passes():
                    w = wl if wlo else wh
                    nc.tensor.matmul(qps[:, :SQ], lhsT=w[:, s2, :],
                                     rhs=xs[:, s2, :],
                                     start=(first and i == 0),
                                     stop=(i == DC // 2 - 1 and wlo == 1),
                                     perf_mode=DR)
            rope_chunk(qps[:, :SQ], qT[:, pair, :])

        # ---- K projection + RoPE -> AllGather (before everything else) ----
        kT_own = sb.tile([P, KFC, SQ], BF, tag="own4", name="kT_own")
        for fc in range(KFC):
            ps = pp.tile([P, 512], FP, tag="pp", name="kps")
            fs = slice(fc * P, (fc + 1) * P)
            # pass-major: the first pass needs only wkh + the xh chunks, so
            # the PE starts before wkl/xl even arrive
            for xs, w, first, last in ((xTh, wkh, True, False),
                                       (xTh, wkl, False, False),
                                       (xTl, wkh, False, True)):
                for i in range(DC // 2):
                    s2 = slice(2 * i, 2 * i + 2)
                    nc.tensor.matmul(ps[:, :SQ], lhsT=w[:, s2, fs],
                                     rhs=xs[:, s2, :],
                                     start=(first and i == 0),
                                     stop=(last and i == DC // 2 - 1),
                                     perf_mode=DR)
            rope_chunk(ps[:, :SQ], kT_own[:, fc, :])
        nc.scalar.dma_start(kag_in[:].rearrange("(c p) s -> p c s", p=P),
                            kT_own[:])
        if solo:
            for r in range(GPB):
                nc.scalar.dma_start(kag_out[r * KF:(r + 1) * KF, :], kag_in[:])
        else:
            nc.gpsimd.collective_compute(
                "AllGather", mybir.AluOpType.bypass, replica_groups=groups,
                ins=[kag_in[:]], outs=[kag_out[:]])

        # ---- land gathered K (kT reuses the wk slot) while Q0 projects ----
        kT = early.tile([P, KFC, S], BF, tag="wkh", name="kT")
        kag_v = kag_out[:].rearrange("(r c p) s -> c p r s", r=GPB, p=P)
        for fc in range(KFC):
            nc.scalar.dma_start(
                kT[:, fc, :].rearrange("p (r s) -> p r s", r=GPB), kag_v[fc])
        qproj(0)

        # ---- V projection -> AllGather (x is the stationary side) ----
        v_own = sb.tile([P, RQ, KF], BF, tag="own4", name="v_own")
        for pc in range(RQ):
            ps = pp.tile([P, 512], FP, tag="pp", name="vps")
            pcs = slice(pc * P, (pc + 1) * P)
            for i in range(DC // 2):
                s2 = slice(2 * i, 2 * i + 2)
                for xs, wlo, first in comp_passes():
                    w = wvl if wlo else wvh
                    nc.tensor.matmul(ps[:, :KF], lhsT=xs[:, s2, pcs],
                                     rhs=w[:, s2, :],
                                     start=(first and i == 0),
                                     stop=(i == DC // 2 - 1 and wlo == 1),
                                     perf_mode=DR)
            nc.vector.tensor_scalar_mul(v_own[:, pc, :], ps[:, :KF], DSC)
            nc.sync.dma_start(vag_in[pc * P:(pc + 1) * P, :], v_own[:, pc, :])
        if solo:
            for r in range(GPB):
                nc.sync.dma_start(vag_out[r * SQ:(r + 1) * SQ, :], vag_in[:])
        else:
            nc.gpsimd.collective_compute(
                "AllGather", mybir.AluOpType.bypass, replica_groups=groups,
                ins=[vag_in[:]], outs=[vag_out[:]])

        qproj(1)

        v_aug = early.tile([P, NKV, SC, VW], BF, tag="wvh", name="v_aug")
        # only the ones-col needs the memset; the DMAs below fill cols 0:HD
        nc.gpsimd.memset(v_aug[:, :, :, HD:HD + 1], 1.0 / XS)
        for c in range(SC):
            nc.sync.dma_start(
                v_aug[:, :, c, 0:HD],
                vag_out[c * P:(c + 1) * P, :].rearrange("p (kv d) -> p kv d", d=HD))

        # ---- per-pair: Q proj + attention; prev group's out-proj interleaved ----
        oT_tiles = {}

        def wo_load(g, nf, tail=False):
            wo_nf = opool.tile([P, 4, 512], BF, tag="wo", name="wo_nf")
            # tail loads go on the swdge queue: the sync queue head-of-line
            # blocks on the last pair's transposes right then
            eng = nc.gpsimd if tail else nc.sync
            eng.dma_start(wo_nf[:], wo_e[g, nf])
            return wo_nf

        out_acc = sb.tile([P, RQ, D], FP)

        def out_proj_m(g, nf, wo_nf, m):
            """Emit one [128-row, 512-col] tile of group g's out-projection.
            Groups 0-2 accumulate (descaled by 1/16) into bf16 SBUF; group 3
            folds the running accumulator back in with a 16*I matmul, then
            the finishing descale-copy alternates ACT/DVE to halve the tail."""
            oT = oT_tiles[g]
            ms = slice(m * P, (m + 1) * P)
            ps = pp.tile([P, 512], FP, tag="pp", name="ops")
            for ch in range(4):
                nc.tensor.matmul(ps[:], lhsT=oT[:, ch, ms],
                                 rhs=wo_nf[:, ch, :],
                                 start=(ch == 0), stop=(ch == 3))
            acc = out_acc[:, m, nf * 512:(nf + 1) * 512]
            if g == 0:
                nc.vector.tensor_scalar_mul(acc, ps[:], ODSC)
            else:
                nc.vector.affine_then_add(acc, ps[:], acc, ODSC, 0.0)
            if g == 3:
                eng = nc.sync if (nf * RQ + m) % 2 else nc.gpsimd
                eng.dma_start(
                    out_e[m * P:(m + 1) * P, nf * 512:(nf + 1) * 512], acc)

        wo3_tiles = {}
        for g in range(4):                    # 4 groups of 4 pairs
            oT_tiles[g] = otp.tile([P, RQ, SQ], BF, tag="oT", name=f"oT_{g}")
            for pi in range(4):               # pairs within group
                pair = g * 4 + pi
                wo_cur = [None]
                kc = pair % 4                 # kv chunk holding both kv heads
                kva, kvb = 2 * (pair % 4), 2 * (pair % 4) + 1

                # seq-major attn.V: per head one psum bank holding 4 q-block
                # accumulators [128 q, 64 v + 1 ones]; col 64 collects the
                # softmax denominator per q row.  Exactly one start=True per
                # bank (the hw zero-region is bank-granular); every other
                # accumulator rides the same lazy zero fill.
                poA = po.tile([P, RQ, VW], FP, tag="poA", name="poA")
                poB = po.tile([P, RQ, VW], FP, tag="poB", name="poB")
                eabs = {}
                for c in range(SC + 3):
                    if c < SC:
                        # scores for both heads of the pair into one 2-bank
                        # psum tile; one exp op covers A and B
                        psAB = psc.tile([P, 1024], FP, tag="psc", name="psAB")
                        nc.tensor.matmul(psAB[:, 0:SQ],
                                         lhsT=kT[0:64, kc, c * P:(c + 1) * P],
                                         rhs=qT[0:64, pair, :],
                                         start=True, stop=True,
                                         tile_position=(0, 0))
                        nc.tensor.matmul(psAB[:, SQ:2 * SQ],
                                         lhsT=kT[64:128, kc, c * P:(c + 1) * P],
                                         rhs=qT[64:128, pair, :],
                                         start=True, stop=True,
                                         tile_position=(64, 0))
                        eab = epool.tile([P, 2, SQ], BF, tag="exp", name="eab")
                        nc.scalar.activation(eab[:], psAB[:], EXPF, scale=EXP_SCALE)
                        eabs[c] = eab
                    if c >= 3:
                        cc_ = c - 3      # attn.V lags three chunks behind exp
                        eab = eabs.pop(cc_)
                        for h, po_t, kvh in ((0, poA, kva), (1, poB, kvb)):
                            for qb in range(RQ):
                                nc.tensor.matmul(
                                    po_t[:, qb, :],
                                    lhsT=eab[:, h, qb * P:(qb + 1) * P],
                                    rhs=v_aug[:, kvh, cc_, :],
                                    start=(cc_ == 0 and qb == 0),
                                    stop=(cc_ == SC - 1),
                                    skip_group_check=True)
                    # spread next-pair q-proj and prev-group out-proj through
                    # the chunk loop so the PE never bunches them at the
                    # pair boundary (ACT rides its 1-chunk buffer)
                    if c == 1 and pair < 14:
                        wq_tiles[pair + 2] = wq_load(pair + 2)
                    if c == 3 and g >= 1:
                        wo_cur[0] = wo_load(g - 1, pi)
                    if c == 5 and pair < 14:
                        qproj(pair + 2)
                    if c == 7 and g == 3 and pi >= 2:
                        wo3_tiles[pi - 2] = wo_load(3, pi - 2)
                    if c in (9, 11, 13, 15) and g >= 1:
                        out_proj_m(g - 1, pi, wo_cur[0], (c - 9) // 2)
                # normalize (per-q denominator is a per-partition scalar;
                # the ones-col held 1/16 so rbc = 16/den and o_n = 16*o)
                # then transpose [q, (h d)] -> [(h d), q] on the DMA xbar
                o_n = onp.tile([P, RQ, 2, HD], BF, tag="on", name="o_n")
                rbcA = npool.tile([P, RQ], FP, tag="rbc", name="rbcA")
                rbcB = npool.tile([P, RQ], FP, tag="rbc", name="rbcB")
                nc.vector.reciprocal(rbcA[:], poA[:, :, HD:HD + 1])
                nc.vector.reciprocal(rbcB[:], poB[:, :, HD:HD + 1])
                for h, po_t, rbc in ((0, poA, rbcA), (1, poB, rbcB)):
                    nc.vector.tensor_mul(
                        o_n[:, :, h, :], po_t[:, :, 0:HD],
                        rbc[:].rearrange("p q -> p q ()").broadcast_to(
                            (P, RQ, HD)))
                last_pair = pair == NQ // 2 - 1
                for qb in range(RQ):
                    # scalar queue only for the last pair (ACT is done with
                    # exps there; mid-kernel it would stall exp dispatch)
                    eng = nc.scalar if last_pair and qb % 2 else nc.sync
                    eng.dma_start_transpose(
                        oT_tiles[g][:, pi, qb * P:(qb + 1) * P],
                        o_n[:, qb, :, :])

            if g == 3:
                for nf in range(4):
                    if nf + 2 < 4:
                        wo3_tiles[nf + 2] = wo_load(3, nf + 2, tail=True)
                    for m in range(RQ):
                        out_proj_m(3, nf, wo3_tiles[nf], m)

    nc.compile()
    return nc


_NC = None


def _get_nc():
    global _NC
    if _NC is None:
        _NC = build()
    return _NC


def _split8(a, scale):
    """Scaled fp8 hi/lo split: a*scale == hi + lo to ~11 mantissa bits."""
    s = (a * scale).astype(np.float32)
    hi = s.astype(F8H)
    lo = (s - hi.astype(np.float32)).astype(F8H)
    return np.ascontiguousarray(hi), np.ascontiguousarray(lo)


def _host_prep(inputs):
    """Swizzle all weights into the on-chip layouts (so device DMAs are
    linear), pre-split everything into scaled fp8 hi/lo pairs, build the
    interleaved-layout CC/SS tables, slice per-core shards.  Q/K features
    keep their natural interleaved order (e0 o0 e1 o1 ...): the RoPE partner
    swap is then an adjacent-partition stream_shuffle on the device."""
    x = np.asarray(inputs["x"], np.float32)
    cos = np.asarray(inputs["cos"], np.float32)
    sin = np.asarray(inputs["sin"], np.float32)
    wq = np.asarray(inputs["wq"], np.float32)
    wk = np.asarray(inputs["wk"], np.float32)
    wv = np.asarray(inputs["wv"], np.float32)
    wo = np.asarray(inputs["wo"], np.float32)

    # device layouts (f32, split to fp8 at the end)
    wq_dev = np.ascontiguousarray(
        wq.reshape(DC, P, DC, P).transpose(2, 1, 0, 3))
    wk_dev = np.ascontiguousarray(
        wk.reshape(DC, P, KF).transpose(1, 0, 2))
    wv_dev = np.ascontiguousarray(
        wv.reshape(DC, P, KF).transpose(1, 0, 2))
    wo_dev = np.ascontiguousarray(
        wo.reshape(RQ, RQ, P, RQ, 512).transpose(0, 3, 2, 1, 4))

    wqh, wql = _split8(wq_dev, WS)
    wkh, wkl = _split8(wk_dev, WS)
    wvh, wvl = _split8(wv_dev, WS)
    wo_bf = np.ascontiguousarray(wo_dev.astype(ml_dtypes.bfloat16))

    cosT = np.ascontiguousarray(cos.T)            # [32, S]
    sinT = np.ascontiguousarray(sin.T)
    cos2 = np.repeat(cosT, 2, axis=0)             # [64, S] rows c0 c0 c1 c1..
    sin2 = np.repeat(sinT, 2, axis=0)
    sign = np.tile(np.array([-1.0, 1.0], np.float32), 32)[:, None]
    CC = np.tile(cos2, (2, 1)) * DSC              # [128, S]; fp8 descale
    SS = np.tile(sin2 * sign, (2, 1)) * DSC

    id8k = np.ascontiguousarray(
        np.eye(P, dtype=np.float32) * XS).astype(ml_dtypes.bfloat16)

    in_maps = []
    for c in range(NCORES):
        b, q = c // GPB, c % GPB
        sl = slice(q * SQ, (q + 1) * SQ)
        x_dev = np.ascontiguousarray(
            x[b, sl, :].T.reshape(DC, P, SQ).transpose(1, 0, 2))
        xh, xl = _split8(x_dev, XS)
        in_maps.append({
            "xh": xh, "xl": xl,
            "wqh": wqh, "wql": wql, "wkh": wkh, "wkl": wkl,
            "wvh": wvh, "wvl": wvl, "wo": wo_bf,
            "cc": np.ascontiguousarray(CC[:, sl]),
            "ss": np.ascontiguousarray(SS[:, sl]),
            "id8k": id8k,
        })
    return in_maps


def kernel(**inputs):
    nc = _get_nc()
    in_maps = _host_prep(inputs)
    res = run_bass_kernel_spmd(nc, in_maps, core_ids=list(range(NCORES)))
    out = np.empty((B, S, D), np.float32)
    for c in range(NCORES):
        b, q = c // GPB, c % GPB
        out[b, q * SQ:(q + 1) * SQ, :] = res.results[c]["out"]
    return out


# revision 76
# speedup vs baseline: 1.0358x; 1.0000x over previous
"""Distributed GQA attention (llama-style RoPE) for one TRN2 chip (8 NeuronCores).

Sharding: core c handles batch b=c//4 and sequence-quarter q=c%4 (512 q-rows).
Each core projects Q for its own rows (all 32 heads), projects K/V for its own
512 positions, AllGathers K/V within its 4-core batch group, runs attention for
its rows, and applies the output projection. Output rows are disjoint across
cores, so no all-reduce is needed; the host concatenates.

On-chip dataflow (per core):
  xT    = x.T, shipped from host pre-split into fp8 hi+lo (x16 scale)
  Q/K/V/out projections run as compensated-fp8 DoubleRow matmuls:
  3 passes (hi*hi + lo*hi + hi*lo) of half-cost fp8 matmuls == 0.75x the
  bf16 cycle count at slightly BETTER-than-bf16 accuracy.  Weight scale
  (x512) and activation scale (x16) are divided back out in the psum
  consumers (RoPE tables, v copy, the softmax ones-column, out accumulate).
  per head pair (with the previous group's out-proj interleaved):
    qT   = wq.T @ xT -> RoPE -> bf16                  [feat, sq]
    sT   = kT_h.T @ qT_h  (row-packed pairs)          [skv, sq]  psum f32
    e    = exp(sT/8) on ScalarE -> bf16
    o    = eab.T @ [v|1/16] seq-major, accumulated    [sq=4x128, 65] psum
           (col 64 collects the softmax denominator per q row; one
            start=True per psum bank, hw zero-region is bank-granular)
    o_n  = o * (16/den)  (per-partition scalar)  -> bf16 (=16*o)
    oT   = dma-xbar transpose of o_n -> [feat, sq]
  out  = oT.T @ wo (bf16), 4 head-group partials summed in f32 SBUF via
         tensor_scalar / affine_then_add with the 1/16 descale folded in.

All weights are pre-swizzled on the HOST into the exact SBUF layouts so every
DMA is a fully-linear copy.  Q/K features stay in natural interleaved order
(e0 o0 e1 o1 ...): the RoPE partner swap is an adjacent-partition
stream_shuffle on the DVE, and the cos/sin tables are shipped row-duplicated
and sign-baked as CC/SS [128, sq] with the fp8 descale folded in.
"""
import sys

sys.path.insert(0, "/opt/trn_rl_repo")

import numpy as np
import ml_dtypes
from contextlib import ExitStack

import concourse.bass as bass
import concourse.mybir as mybir
import concourse.tile as tile
from concourse import bacc
from concourse.bass_utils import run_bass_kernel_spmd
from concourse.masks import make_identity

B, S, D = 2, 2048, 2048
NQ, NKV, HD = 32, 8, 64
NCORES = 8
GPB = 4                 # cores per batch group
SQ = S // GPB           # 512 q-rows per core
P = 128
DC = D // P             # 16 contraction chunks
KF = NKV * HD           # 512 kv feature dim
KFC = KF // P           # 4 kv feature chunks
SC = S // P             # 16 skv chunks
RQ = SQ // P            # 4 q-row blocks
VW = HD + 1             # v_aug width: 64 v cols + 1 ones col (denominator)

FP = mybir.dt.float32
BF = mybir.dt.bfloat16
F8 = mybir.dt.float8e4
F8H = ml_dtypes.float8_e4m3
DR = mybir.MatmulPerfMode.DoubleRow
EXPF = mybir.ActivationFunctionType.Exp
COPYF = mybir.ActivationFunctionType.Copy
EXP_SCALE = 1.0 / 8.0   # 1/sqrt(HD)

XS = 16.0               # fp8 pre-scale on activations
WS = 512.0              # fp8 pre-scale on weights
DSC = 1.0 / (XS * WS)   # descale folded into psum consumers
ODSC = 1.0 / XS         # out-proj descale (oT carries x16; wo is bf16)


def build(solo=False):
    nc = bacc.Bacc("TRN2", target_bir_lowering=False, debug=False,
                   num_devices=1 if solo else NCORES)

    xh_e = nc.dram_tensor("xh", [P, DC, SQ], F8, kind="ExternalInput").ap()
    xl_e = nc.dram_tensor("xl", [P, DC, SQ], F8, kind="ExternalInput").ap()
    wqh_e = nc.dram_tensor("wqh", [DC, P, DC, P], F8, kind="ExternalInput").ap()
    wql_e = nc.dram_tensor("wql", [DC, P, DC, P], F8, kind="ExternalInput").ap()
    wkh_e = nc.dram_tensor("wkh", [P, DC, KF], F8, kind="ExternalInput").ap()
    wkl_e = nc.dram_tensor("wkl", [P, DC, KF], F8, kind="ExternalInput").ap()
    wvh_e = nc.dram_tensor("wvh", [P, DC, KF], F8, kind="ExternalInput").ap()
    wvl_e = nc.dram_tensor("wvl", [P, DC, KF], F8, kind="ExternalInput").ap()
    wo_e = nc.dram_tensor("wo", [RQ, RQ, P, RQ, 512], BF, kind="ExternalInput").ap()
    cc_e = nc.dram_tensor("cc", [P, SQ], FP, kind="ExternalInput").ap()
    ss_e = nc.dram_tensor("ss", [P, SQ], FP, kind="ExternalInput").ap()
    id8k_e = nc.dram_tensor("id8k", [P, P], BF, kind="ExternalInput").ap()
    out_e = nc.dram_tensor("out", [SQ, D], FP, kind="ExternalOutput").ap()

    groups = [[0, 1, 2, 3], [4, 5, 6, 7]]

    with tile.TileContext(nc) as tc, ExitStack() as ctx:
        sb = ctx.enter_context(tc.tile_pool(name="sb", bufs=1))
        rp = ctx.enter_context(tc.tile_pool(name="rp", bufs=3))
        epool = ctx.enter_context(tc.tile_pool(name="epool", bufs=8))
        npool = ctx.enter_context(tc.tile_pool(name="npool", bufs=3))
        onp = ctx.enter_context(tc.tile_pool(name="onp", bufs=3))
        opool = ctx.enter_context(tc.tile_pool(name="opool", bufs=3))
        otp = ctx.enter_context(tc.tile_pool(name="otp", bufs=2))
        early = ctx.enter_context(tc.tile_pool(name="early", bufs=1))
        wqp = ctx.enter_context(tc.tile_pool(name="wqp", bufs=3))
        dram = ctx.enter_context(tc.tile_pool(name="dram", bufs=1, space="DRAM"))
        pp = ctx.enter_context(tc.tile_pool(name="pp", bufs=2, space="PSUM"))
        psc = ctx.enter_context(tc.tile_pool(name="psc", bufs=2, space="PSUM"))
        po = ctx.enter_context(tc.tile_pool(name="po", bufs=1, space="PSUM"))

        # ---- constants ----
        cc_sb = sb.tile([P, SQ], FP)
        ss_sb = sb.tile([P, SQ], FP)
        id8k = sb.tile([P, P], BF)      # 8192 * identity (descale-matched)

        # features stay in natural interleaved order (e0 o0 e1 o1 ...): the
        # RoPE partner swap is adjacent-partition, expressible as an intra-
        # quadrant stream_shuffle; score contraction is order-invariant.
        SWAP_MASK = [i ^ 1 for i in range(32)]

        def rope_chunk(ps, dst):
            """dst = RoPE(ps)*DSC in interleaved layout; ps [128,SQ] psum."""
            t0 = rp.tile([P, SQ], FP, tag="t0")
            tsh = rp.tile([P, SQ], FP, tag="tsh")
            t1 = rp.tile([P, SQ], FP, tag="t1")
            nc.vector.tensor_mul(t0[:], ps[:], cc_sb[:])
            nc.vector.stream_shuffle(tsh[:], ps[:], SWAP_MASK)
            nc.vector.tensor_mul(t1[:], tsh[:], ss_sb[:])
            nc.vector.tensor_add(dst, t0[:], t1[:])

        qT = sb.tile([P, DC, SQ], BF)
        kag_in = dram.tile([KF, SQ], BF)
        kag_out = dram.tile([GPB * KF, SQ], BF)
        vag_in = dram.tile([SQ, KF], BF)
        vag_out = dram.tile([S, KF], BF)

        # ---- pure input loads first, split across queues: wq + x on the
        #      sync queue, the K/V/id weights on the (early-idle) scalar
        #      queue, wo on the gpsimd swdge queue later ----
        wq_tiles = {}

        def wq_load(pair):
            wh = wqp.tile([P, DC, P], F8, tag="wqh", name=f"wqh_{pair}")
            wl = wqp.tile([P, DC, P], F8, tag="wql", name=f"wql_{pair}")
            nc.sync.dma_start(wh[:], wqh_e[pair])
            nc.sync.dma_start(wl[:], wql_e[pair])
            return wh, wl

        # one queue, strict consumption order: the DMA pipe is a single
        # serialized resource in practice, so emission order = arrival order.
        # K comes FIRST so the AllGather (the longest dependency chain of
        # the attention phase) is in flight as early as possible.
        xTh = early.tile([P, DC, SQ], F8, tag="xTh", name="xTh")
        xTl = early.tile([P, DC, SQ], F8, tag="xTl", name="xTl")
        wkh = early.tile([P, DC, KF], F8, tag="wkh", name="wkh")
        wkl = early.tile([P, DC, KF], F8, tag="wkl", name="wkl")
        wvh = early.tile([P, DC, KF], F8, tag="wvh", name="wvh")
        wvl = early.tile([P, DC, KF], F8, tag="wvl", name="wvl")

        def x_load(xc):
            s4 = slice(4 * xc, 4 * (xc + 1))
            nc.sync.dma_start(xTh[:, s4, :], xh_e[:, s4, :])
            nc.sync.dma_start(xTl[:, s4, :], xl_e[:, s4, :])

        nc.sync.dma_start(wkh[:, 0:8, :], wkh_e[:, 0:8, :])
        nc.sync.dma_start(xTh[:, 0:4, :], xh_e[:, 0:4, :])
        nc.sync.dma_start(wkh[:, 8:16, :], wkh_e[:, 8:16, :])
        for xc in range(1, 4):
            s4 = slice(4 * xc, 4 * (xc + 1))
            nc.sync.dma_start(xTh[:, s4, :], xh_e[:, s4, :])
        nc.sync.dma_start(wkl[:], wkl_e)
        for xc in range(4):
            s4 = slice(4 * xc, 4 * (xc + 1))
            nc.sync.dma_start(xTl[:, s4, :], xl_e[:, s4, :])
        nc.sync.dma_start(cc_sb[:], cc_e)
        nc.sync.dma_start(ss_sb[:], ss_e)
        wq_tiles[0] = wq_load(0)
        nc.sync.dma_start(wvh[:], wvh_e)
        nc.sync.dma_start(wvl[:], wvl_e)
        wq_tiles[1] = wq_load(1)
        nc.sync.dma_start(id8k[:], id8k_e)

        def comp_passes():
            """(x, w, first) triples for the 3 compensated DR passes."""
            return ((xTh, 0, True), (xTl, 0, False), (xTh, 1, False))

        def qproj(pair):
            wh, wl = wq_tiles.pop(pair)
            qps = pp.tile([P, 512], FP, tag="pp", name="qps")
            for i in range(DC // 2):
                s2 = slice(2 * i, 2 * i + 2)
                for xs, wlo, first in comp_passes():
                    w = wl if wlo else wh
                    nc.tensor.matmul(qps[:, :SQ], lhsT=w[:, s2, :],
                                     rhs=xs[:, s2, :],
                                     start=(first and i == 0),
                                     stop=(i == DC // 2 - 1 and wlo == 1),
                                     perf_mode=DR)
            rope_chunk(qps[:, :SQ], qT[:, pair, :])

        # ---- K projection + RoPE -> AllGather (before everything else) ----
        kT_own = sb.tile([P, KFC, SQ], BF, tag="own4", name="kT_own")
        for fc in range(KFC):
            ps = pp.tile([P, 512], FP, tag="pp", name="kps")
            fs = slice(fc * P, (fc + 1) * P)
            # pass-major: the first pass needs only wkh + the xh chunks, so
            # the PE starts before wkl/xl even arrive
            for xs, w, first, last in ((xTh, wkh, True, False),
                                       (xTh, wkl, False, False),
                                       (xTl, wkh, False, True)):
                for i in range(DC // 2):
                    s2 = slice(2 * i, 2 * i + 2)
                    nc.tensor.matmul(ps[:, :SQ], lhsT=w[:, s2, fs],
                                     rhs=xs[:, s2, :],
                                     start=(first and i == 0),
                                     stop=(last and i == DC // 2 - 1),
                                     perf_mode=DR)
            rope_chunk(ps[:, :SQ], kT_own[:, fc, :])
        nc.scalar.dma_start(kag_in[:].rearrange("(c p) s -> p c s", p=P),
                            kT_own[:])
        if solo:
            for r in range(GPB):
                nc.scalar.dma_start(kag_out[r * KF:(r + 1) * KF, :], kag_in[:])
        else:
            nc.gpsimd.collective_compute(
                "AllGather", mybir.AluOpType.bypass, replica_groups=groups,
                ins=[kag_in[:]], outs=[kag_out[:]])

        # ---- land gathered K (kT reuses the wk slot) while Q0 projects ----
        kT = early.tile([P, KFC, S], BF, tag="wkh", name="kT")
        kag_v = kag_out[:].rearrange("(r c p) s -> c p r s", r=GPB, p=P)
        for fc in range(KFC):
            nc.scalar.dma_start(
                kT[:, fc, :].rearrange("p (r s) -> p r s", r=GPB), kag_v[fc])
        qproj(0)

        # ---- V projection -> AllGather (x is the stationary side) ----
        v_own = sb.tile([P, RQ, KF], BF, tag="own4", name="v_own")
        for pc in range(RQ):
            ps = pp.tile([P, 512], FP, tag="pp", name="vps")
            pcs = slice(pc * P, (pc + 1) * P)
            for i in range(DC // 2):
                s2 = slice(2 * i, 2 * i + 2)
                for xs, wlo, first in comp_passes():
                    w = wvl if wlo else wvh
                    nc.tensor.matmul(ps[:, :KF], lhsT=xs[:, s2, pcs],
                                     rhs=w[:, s2, :],
                                     start=(first and i == 0),
                                     stop=(i == DC // 2 - 1 and wlo == 1),
                                     perf_mode=DR)
            nc.vector.tensor_scalar_mul(v_own[:, pc, :], ps[:, :KF], DSC)
            nc.sync.dma_start(vag_in[pc * P:(pc + 1) * P, :], v_own[:, pc, :])
        if solo:
            for r in range(GPB):
                nc.sync.dma_start(vag_out[r * SQ:(r + 1) * SQ, :], vag_in[:])
        else:
            nc.gpsimd.collective_compute(
                "AllGather", mybir.AluOpType.bypass, replica_groups=groups,
                ins=[vag_in[:]], outs=[vag_out[:]])

        qproj(1)

        v_aug = early.tile([P, NKV, SC, VW], BF, tag="wvh", name="v_aug")
        # only the ones-col needs the memset; the DMAs below fill cols 0:HD
        nc.gpsimd.memset(v_aug[:, :, :, HD:HD + 1], 1.0 / XS)
        for c in range(SC):
            nc.sync.dma_start(
                v_aug[:, :, c, 0:HD],
                vag_out[c * P:(c + 1) * P, :].rearrange("p (kv d) -> p kv d", d=HD))

        # ---- per-pair: Q proj + attention; prev group's out-proj interleaved ----
        oT_tiles = {}

        def wo_load(g, nf, tail=False):
            wo_nf = opool.tile([P, 4, 512], BF, tag="wo", name="wo_nf")
            # tail loads go on the swdge queue: the sync queue head-of-line
            # blocks on the last pair's transposes right then
            eng = nc.gpsimd if tail else nc.sync
            eng.dma_start(wo_nf[:], wo_e[g, nf])
            return wo_nf

        out_acc = sb.tile([P, RQ, D], FP)

        def out_proj_m(g, nf, wo_nf, m):
            """Emit one [128-row, 512-col] tile of group g's out-projection.
            Groups 0-2 accumulate (descaled by 1/16) into bf16 SBUF; group 3
            folds the running accumulator back in with a 16*I matmul, then
            the finishing descale-copy alternates ACT/DVE to halve the tail."""
            oT = oT_tiles[g]
            ms = slice(m * P, (m + 1) * P)
            ps = pp.tile([P, 512], FP, tag="pp", name="ops")
            for ch in range(4):
                nc.tensor.matmul(ps[:], lhsT=oT[:, ch, ms],
                                 rhs=wo_nf[:, ch, :],
                                 start=(ch == 0), stop=(ch == 3))
            acc = out_acc[:, m, nf * 512:(nf + 1) * 512]
            if g == 0:
                nc.vector.tensor_scalar_mul(acc, ps[:], ODSC)
            else:
                nc.vector.affine_then_add(acc, ps[:], acc, ODSC, 0.0)
            if g == 3:
                eng = nc.sync if (nf * RQ + m) % 2 else nc.gpsimd
                eng.dma_start(
                    out_e[m * P:(m + 1) * P, nf * 512:(nf + 1) * 512], acc)

        wo3_tiles = {}
        for g in range(4):                    # 4 groups of 4 pairs
            oT_tiles[g] = otp.tile([P, RQ, SQ], BF, tag="oT", name=f"oT_{g}")
            for pi in range(4):               # pairs within group
                pair = g * 4 + pi
                wo_cur = [None]
                kc = pair % 4                 # kv chunk holding both kv heads
                kva, kvb = 2 * (pair % 4), 2 * (pair % 4) + 1

                # seq-major attn.V: per head one psum bank holding 4 q-block
                # accumulators [128 q, 64 v + 1 ones]; col 64 collects the
                # softmax denominator per q row.  Exactly one start=True per
                # bank (the hw zero-region is bank-granular); every other
                # accumulator rides the same lazy zero fill.
                poA = po.tile([P, RQ, VW], FP, tag="poA", name="poA")
                poB = po.tile([P, RQ, VW], FP, tag="poB", name="poB")
                eabs = {}
                for c in range(SC + 3):
                    if c < SC:
                        # scores for both heads of the pair into one 2-bank
                        # psum tile; one exp op covers A and B
                        psAB = psc.tile([P, 1024], FP, tag="psc", name="psAB")
                        nc.tensor.matmul(psAB[:, 0:SQ],
                                         lhsT=kT[0:64, kc, c * P:(c + 1) * P],
                                         rhs=qT[0:64, pair, :],
                                         start=True, stop=True,
                                         tile_position=(0, 0))
                        nc.tensor.matmul(psAB[:, SQ:2 * SQ],
                                         lhsT=kT[64:128, kc, c * P:(c + 1) * P],
                                         rhs=qT[64:128, pair, :],
                                         start=True, stop=True,
                                         tile_position=(64, 0))
                        eab = epool.tile([P, 2, SQ], BF, tag="exp", name="eab")
                        nc.scalar.activation(eab[:], psAB[:], EXPF, scale=EXP_SCALE)
                        eabs[c] = eab
                    if c >= 3:
                        cc_ = c - 3      # attn.V lags three chunks behind exp
                        eab = eabs.pop(cc_)
                        for h, po_t, kvh in ((0, poA, kva), (1, poB, kvb)):
                            for qb in range(RQ):
                                nc.tensor.matmul(
                                    po_t[:, qb, :],
                                    lhsT=eab[:, h, qb * P:(qb + 1) * P],
                                    rhs=v_aug[:, kvh, cc_, :],
                                    start=(cc_ == 0 and qb == 0),
                                    stop=(cc_ == SC - 1),
                                    skip_group_check=True)
                    # spread next-pair q-proj and prev-group out-proj through
                    # the chunk loop so the PE never bunches them at the
                    # pair boundary (ACT rides its 1-chunk buffer)
                    if c == 1 and pair < 14:
                        wq_tiles[pair + 2] = wq_load(pair + 2)
                    if c == 3 and g >= 1:
                        wo_cur[0] = wo_load(g - 1, pi)
                    if c == 5 and pair < 14:
                        qproj(pair + 2)
                    if c == 7 and g == 3 and pi >= 2:
                        wo3_tiles[pi - 2] = wo_load(3, pi - 2)
                    if c in (9, 11, 13, 15) and g >= 1:
                        out_proj_m(g - 1, pi, wo_cur[0], (c - 9) // 2)
                # normalize (per-q denominator is a per-partition scalar;
                # the ones-col held 1/16 so rbc = 16/den and o_n = 16*o)
                # then transpose [q, (h d)] -> [(h d), q] on the DMA xbar
                o_n = onp.tile([P, RQ, 2, HD], BF, tag="on", name="o_n")
                rbcA = npool.tile([P, RQ], FP, tag="rbc", name="rbcA")
                rbcB = npool.tile([P, RQ], FP, tag="rbc", name="rbcB")
                nc.vector.reciprocal(rbcA[:], poA[:, :, HD:HD + 1])
                nc.vector.reciprocal(rbcB[:], poB[:, :, HD:HD + 1])
                for h, po_t, rbc in ((0, poA, rbcA), (1, poB, rbcB)):
                    nc.vector.tensor_mul(
                        o_n[:, :, h, :], po_t[:, :, 0:HD],
                        rbc[:].rearrange("p q -> p q ()").broadcast_to(
                            (P, RQ, HD)))
                last_pair = pair == NQ // 2 - 1
                for qb in range(RQ):
                    # scalar queue only for the last pair (ACT is done with
                    # exps there; mid-kernel it would stall exp dispatch)
                    eng = nc.scalar if last_pair and qb % 2 else nc.sync
                    eng.dma_start_transpose(
                        oT_tiles[g][:, pi, qb * P:(qb + 1) * P],
                        o_n[:, qb, :, :])

            if g == 3:
                for nf in range(4):
                    if nf + 2 < 4:
                        wo3_tiles[nf + 2] = wo_load(3, nf + 2, tail=True)
                    for m in range(RQ):
                        out_proj_m(3, nf, wo3_tiles[nf], m)

    nc.compile()
    return nc


_NC = None


def _get_nc():
    global _NC
    if _NC is None:
        _NC = build()
    return _NC


def _split8(a, scale):
    """Scaled fp8 hi/lo split: a*scale == hi + lo to ~11 mantissa bits."""
    s = (a * scale).astype(np.float32)
    hi = s.astype(F8H)
    lo = (s - hi.astype(np.float32)).astype(F8H)
    return np.ascontiguousarray(hi), np.ascontiguousarray(lo)


def _host_prep(inputs):
    """Swizzle all weights into the on-chip layouts (so device DMAs are
    linear), pre-split everything into scaled fp8 hi/lo pairs, build the
    interleaved-layout CC/SS tables, slice per-core shards.  Q/K features
    keep their natural interleaved order (e0 o0 e1 o1 ...): the RoPE partner
    swap is then an adjacent-partition stream_shuffle on the device."""
    x = np.asarray(inputs["x"], np.float32)
    cos = np.asarray(inputs["cos"], np.float32)
    sin = np.asarray(inputs["sin"], np.float32)
    wq = np.asarray(inputs["wq"], np.float32)
    wk = np.asarray(inputs["wk"], np.float32)
    wv = np.asarray(inputs["wv"], np.float32)
    wo = np.asarray(inputs["wo"], np.float32)

    # device layouts (f32, split to fp8 at the end)
    wq_dev = np.ascontiguousarray(
        wq.reshape(DC, P, DC, P).transpose(2, 1, 0, 3))
    wk_dev = np.ascontiguousarray(
        wk.reshape(DC, P, KF).transpose(1, 0, 2))
    wv_dev = np.ascontiguousarray(
        wv.reshape(DC, P, KF).transpose(1, 0, 2))
    wo_dev = np.ascontiguousarray(
        wo.reshape(RQ, RQ, P, RQ, 512).transpose(0, 3, 2, 1, 4))

    wqh, wql = _split8(wq_dev, WS)
    wkh, wkl = _split8(wk_dev, WS)
    wvh, wvl = _split8(wv_dev, WS)
    wo_bf = np.ascontiguousarray(wo_dev.astype(ml_dtypes.bfloat16))

    cosT = np.ascontiguousarray(cos.T)            # [32, S]
    sinT = np.ascontiguousarray(sin.T)
    cos2 = np.repeat(cosT, 2, axis=0)             # [64, S] rows c0 c0 c1 c1..
    sin2 = np.repeat(sinT, 2, axis=0)
    sign = np.tile(np.array([-1.0, 1.0], np.float32), 32)[:, None]
    CC = np.tile(cos2, (2, 1)) * DSC              # [128, S]; fp8 descale
    SS = np.tile(sin2 * sign, (2, 1)) * DSC

    id8k = np.ascontiguousarray(
        np.eye(P, dtype=np.float32) * XS).astype(ml_dtypes.bfloat16)

    in_maps = []
    for c in range(NCORES):
        b, q = c // GPB, c % GPB
        sl = slice(q * SQ, (q + 1) * SQ)
        x_dev = np.ascontiguousarray(
            x[b, sl, :].T.reshape(DC, P, SQ).transpose(1, 0, 2))
        xh, xl = _split8(x_dev, XS)
        in_maps.append({
            "xh": xh, "xl": xl,
            "wqh": wqh, "wql": wql, "wkh": wkh, "wkl": wkl,
            "wvh": wvh, "wvl": wvl, "wo": wo_bf,
            "cc": np.ascontiguousarray(CC[:, sl]),
            "ss": np.ascontiguousarray(SS[:, sl]),
            "id8k": id8k,
        })
    return in_maps


def kernel(**inputs):
    nc = _get_nc()
    in_maps = _host_prep(inputs)
    res = run_bass_kernel_spmd(nc, in_maps, core_ids=list(range(NCORES)))
    out = np.empty((B, S, D), np.float32)
    for c in range(NCORES):
        b, q = c // GPB, c % GPB
        out[b, q * SQ:(q + 1) * SQ, :] = res.results[c]["out"]
    return out
